# revision 1
# baseline (speedup 1.0000x reference)
"""GAT message-passing layer on 8 Trainium2 NeuronCores (Bass/Tile).

Strategy (matches the sharding hint): nodes are partitioned across the 8
cores; each edge is owned by the core that owns its destination node, so the
segment softmax and the weighted scatter-sum stay core-local.  Every core
computes the bf16 k/v projection table for all nodes (cheap, replicated;
natural-layout loads + PE transposes — no slow DMA-transpose) and keeps q for
its local nodes in SBUF.  Per-edge k rows are fetched feat-major with SWDGE
`dma_gather` (transpose mode), v rows edge-major (row mode); per-edge q is NOT
gathered — within a sub all 128 edges share one dst block, so qT per edge is a
one-hot select matmul q_blk^T @ ST against the streamed ST matrix.  Scores are
reduced on the PE with a block-diagonal head selector, the segment softmax
numerator/denominator are accumulated in PSUM via one-hot scatter matmuls, and
the epilogue (divide, residual, LN, FFN with PReLU folded into two weight
matrices, LN) runs per 128-node block.
"""

import sys

sys.path.insert(0, "/opt/trn_rl_repo")

import math
import os
from contextlib import ExitStack
from dataclasses import dataclass, field

import numpy as np
import ml_dtypes

import concourse.bass as bass
import concourse.bacc as bacc
import concourse.mybir as mybir
import concourse.tile as tile
from concourse._compat import with_exitstack
from concourse.bass_utils import run_bass_kernel_spmd
from concourse.library_config import mlp as mlp_lib

bf16 = ml_dtypes.bfloat16
P = 128
AF = mybir.ActivationFunctionType
OP = mybir.AluOpType
FP32 = mybir.dt.float32
BF16 = mybir.dt.bfloat16
I16 = mybir.dt.int16


@dataclass
class GATCfg:
    n_nodes: int = 50000
    n_edges: int = 640000
    feats: int = 128
    heads: int = 8
    dhead: int = 16
    dff: int = 512
    n_cores: int = 8
    grp: int = 2  # dst blocks per gather group
    wave: int = 4  # 128-edge subchunks per score/message wave
    tmult_chunk: int = 2048
    qsel_chunk: int = 512  # qT select matmul column chunk (1 PSUM bank)
    debug: bool = False

    @property
    def npc(self):  # nodes per core
        return self.n_nodes // self.n_cores

    @property
    def nblk(self):  # local 128-node blocks per core
        return (self.npc + P - 1) // P

    @property
    def local_pad(self):
        return self.nblk * P

    @property
    def npad(self):  # padded global node count (k/v table rows)
        return ((self.n_nodes + P - 1) // P) * P

    @property
    def half(self):  # int16 index split point (row offset base)
        h = self.npad // 2
        assert h < 32768 and (self.npad - h) <= 32768
        return h

    @property
    def ngrp(self):
        return (self.nblk + self.grp - 1) // self.grp


def _wrap16(idx):
    """int16 index list -> [128, n/16] SWDGE layout (16-wrap, replicated x8)."""
    idx = np.asarray(idx, np.int16)
    n = len(idx)
    assert n % 16 == 0
    return np.tile(idx.reshape(n // 16, 16).T, (8, 1)).copy()


def _prep(inputs, cfg: GATCfg):
    """Host-side graph partitioning / padding / index+S-matrix construction."""
    c = cfg
    feat = np.asarray(inputs["feat"], np.float32)
    src = np.asarray(inputs["src"], np.int64)
    dst = np.asarray(inputs["dst"], np.int64)

    feat_pad = np.zeros((c.npad, c.feats), np.float32)
    feat_pad[: c.n_nodes] = feat
    feat16 = feat_pad.astype(bf16)

    # ---- per (core, block, half) edge lists ----
    core_of = dst // c.npc
    per_core = []
    for ci in range(c.n_cores):
        sel = np.nonzero(core_of == ci)[0]
        dloc = dst[sel] - ci * c.npc
        blk = dloc // P
        half = (src[sel] >= c.half).astype(np.int64)
        order = np.lexsort((dloc, half, blk))
        sel, dloc, blk, half = sel[order], dloc[order], blk[order], half[order]
        lists = {}
        for b in range(c.nblk):
            for h in range(2):
                m = (blk == b) & (half == h)
                lists[(b, h)] = (src[sel[m]], dloc[m])
        per_core.append(lists)

    # uniform sub-chunk counts across cores
    n_sub = np.zeros((c.nblk, 2), np.int64)
    for b in range(c.nblk):
        for h in range(2):
            mx = max(len(per_core[ci][(b, h)][0]) for ci in range(c.n_cores))
            n_sub[b, h] = (mx + P - 1) // P

    assert (n_sub > 0).all(), "two-sweep emit needs every (block, half) nonempty"

    # ---- group structure (shared across cores) ----
    groups = []  # list of dicts with static metadata
    scol = 0
    for g in range(c.ngrp):
        bs = list(range(g * c.grp, min((g + 1) * c.grp, c.nblk)))
        L_lo = int(sum(n_sub[b, 0] for b in bs)) * P
        L_hi = int(sum(n_sub[b, 1] for b in bs)) * P
        subs = []
        runs = []  # contiguous (block, col, ncols) spans for the qT select
        # per-block first/last sub bookkeeping (block's subs = its lo + hi subs)
        tot_per_block = {b: int(n_sub[b, 0] + n_sub[b, 1]) for b in bs}
        seen = {b: 0 for b in bs}
        s_idx = 0
        for h in range(2):
            for b in bs:
                ns = int(n_sub[b, h])
                if ns:
                    runs.append(dict(block=b, col=s_idx * P, ncols=ns * P, half=h))
                for _ in range(ns):
                    seen[b] += 1
                    subs.append(
                        dict(
                            block=b,
                            col=s_idx * P,
                            first=seen[b] == 1,
                            last=seen[b] == tot_per_block[b],
                        )
                    )
                    s_idx += 1
        groups.append(
            dict(
                bs=bs, L_lo=L_lo, L_hi=L_hi, L=L_lo + L_hi, subs=subs,
                runs=runs, scol=scol,
            )
        )
        scol += L_lo + L_hi

    tot_cols = scol
    tot_lo = sum(g["L_lo"] for g in groups)
    tot_hi = sum(g["L_hi"] for g in groups)

    meta = dict(groups=groups, tot_cols=tot_cols, tot_lo=tot_lo, tot_hi=tot_hi)

    # ---- per-core streams ----
    per_core_streams = []
    for ci in range(c.n_cores):
        kv_lo = np.zeros(tot_lo, np.int16)
        kv_hi = np.zeros(tot_hi, np.int16)
        S = np.zeros((P, tot_cols), np.float32)
        ST = np.zeros((P, tot_cols), np.float32)
        olo = ohi = 0
        for g in groups:
            gcol = g["scol"]
            i = 0  # edge position within group tile
            for h in range(2):
                for b in g["bs"]:
                    s_arr, d_arr = per_core[ci][(b, h)]
                    npadded = int(n_sub[b, h]) * P
                    rel = np.zeros(npadded, np.int16)
                    rel[: len(s_arr)] = (s_arr - (c.half if h else 0)).astype(
                        np.int16
                    )
                    if h == 0:
                        kv_lo[olo : olo + npadded] = rel
                        olo += npadded
                    else:
                        kv_hi[ohi : ohi + npadded] = rel
                        ohi += npadded
                    # one-hot S: edge j (pos i+j) -> col 128*s + (dloc - b*128)
                    jj = np.arange(len(d_arr))
                    pos = i + jj
                    ss = pos // P
                    pp = pos % P
                    S[pp, gcol + ss * P + (d_arr - b * P)] = 1.0
                    # one-hot ST (transposed layout): row = dst slot, col = pos
                    ST[d_arr - b * P, gcol + pos] = 1.0
                    i += npadded
        feat32_loc = np.zeros((c.local_pad, c.feats), np.float32)
        feat32_loc[: c.npc] = feat[ci * c.npc : (ci + 1) * c.npc]
        per_core_streams.append(
            dict(
                kv_idx_lo=_wrap16(kv_lo),
                kv_idx_hi=_wrap16(kv_hi),
                S_all=S.astype(bf16),
                ST_all=ST.astype(bf16),
                feat32_loc=feat32_loc,
            )
        )

    # ---- shared weight/constant tensors ----
    W1 = np.asarray(inputs["W1"], np.float32)
    W2 = np.asarray(inputs["W2"], np.float32)
    a = np.asarray(inputs["prelu_a"], np.float32)
    # prelu(x) = max(x,0) + a*min(x,0) = ((1+a)/2)*x + ((1-a)/2)*|x|
    nh = c.dff // P
    # [dff, F] -> [P, nh, F] so each head-slice is an SBUF [128 x F] lhsT
    W2a = (
        (((1.0 + a) / 2.0)[:, None] * W2)
        .reshape(nh, P, c.feats)
        .transpose(1, 0, 2)
        .astype(bf16)
    )
    W2b = (
        (((1.0 - a) / 2.0)[:, None] * W2)
        .reshape(nh, P, c.feats)
        .transpose(1, 0, 2)
        .astype(bf16)
    )
    wkv = np.concatenate(
        [np.asarray(inputs["Wk"], np.float32), np.asarray(inputs["Wv"], np.float32)],
        axis=1,
    )
    shared = dict(
        feat16=feat16,
        wq=np.asarray(inputs["Wq"], np.float32).astype(bf16),
        wkv=wkv.astype(bf16),
        w1=W1.astype(bf16),
        w2a=W2a,
        w2b=W2b,
        b1t=np.ascontiguousarray(
            np.asarray(inputs["b1"], np.float32).reshape(nh, P).T
        ),
        b2rep=np.tile(np.asarray(inputs["b2"], np.float32)[None, :], (P, 1)),
        grep=np.tile(np.asarray(inputs["ln1_g"], np.float32)[None, :], (P, 1)),
        brep=np.tile(np.asarray(inputs["ln1_b"], np.float32)[None, :], (P, 1)),
        ident=np.eye(P, dtype=np.float32).astype(bf16),
    )
    return meta, per_core_streams, shared


@with_exitstack
def _emit(ctx: ExitStack, tc: tile.TileContext, t, meta, cfg: GATCfg):
    """Emit the per-core program. `t` maps tensor name -> DRAM AP."""
    c = cfg
    nc = tc.nc
    groups = meta["groups"]
    nh = c.dff // P
    scale = 1.0 / math.sqrt(c.heads * c.dhead)

    with tc.tile_critical():
        nc.gpsimd.load_library(mlp_lib)

    # ---------- persistent pool: constants, indices, q + ft2 storage ----------
    keep = ctx.enter_context(tc.tile_pool(name="keep", bufs=1))

    def load_const(name, shape, dtype):
        tl = keep.tile(shape, dtype, tag=name)
        nc.sync.dma_start(tl[:], t[name][:])
        return tl

    wq = load_const("wq", [P, P], BF16)
    wkv = load_const("wkv", [P, 2 * c.feats], BF16)
    w1 = load_const("w1", [P, c.dff], BF16)
    w2a = load_const("w2a", [P, nh, c.feats], BF16)
    w2b = load_const("w2b", [P, nh, c.feats], BF16)
    b1t = load_const("b1t", [P, nh], FP32)
    b2rep = load_const("b2rep", [P, P], FP32)
    grep = load_const("grep", [P, P], FP32)
    brep = load_const("brep", [P, P], FP32)
    ident = load_const("ident", [P, P], BF16)
    kvlo = load_const("kv_idx_lo", [P, max(meta["tot_lo"] // 16, 1)], I16)
    kvhi = load_const("kv_idx_hi", [P, max(meta["tot_hi"] // 16, 1)], I16)

    q_sb = keep.tile([P, c.nblk, c.feats], BF16, tag="q_sb")
    ftden_sb = keep.tile([P, c.nblk, 136], FP32, tag="ftden_sb")
    ftden_lo = keep.tile([P, c.nblk, 136], BF16, tag="ftden_lo")
    nc.vector.memset(ftden_sb[:], 0.0)
    nc.vector.memset(ftden_lo[:], 0.0)

    dram = ctx.enter_context(tc.tile_pool(name="dram", bufs=1, space="DRAM"))
    kv_table = dram.tile([c.npad, 2 * c.feats], BF16)

    # ---------- phase 1: projection tables (two-sweep: lo half first) ----------
    # natural-layout loads + PE transpose (DMA-transpose is ~4us/tile serialized)
    feat_r = t["feat16"][:].rearrange("(s p) f -> p s f", p=P)
    kv_r = kv_table[:].rearrange("(s p) f -> p s f", p=P)
    floc_r = t["feat16_loc"][:].rearrange("(s p) f -> p s f", p=P)
    PB = 2  # blocks per DMA batch
    nbl = c.npad // P
    lo_blocks = (c.half + P - 1) // P
    assert lo_blocks % PB == 0
    n_pairsA = lo_blocks // PB
    n_kv_stores = (nbl + PB - 1) // PB
    pairsB = list(range(n_pairsA, n_kv_stores))

    def emit_pair(pair, prj):
        prj_ft, prj_tps, prj_tsb, prj_ps, prj_sb = prj
        blks = range(pair * PB, min((pair + 1) * PB, nbl))
        nb_ = len(blks)
        ft = prj_ft.tile([P, PB, P], BF16, tag="ft")
        nc.sync.dma_start(
            ft[:, 0:nb_, :], feat_r[:, pair * PB : pair * PB + nb_, :]
        )
        sb = prj_sb.tile([P, PB, 2 * c.feats], BF16, tag="kvsb")
        for s in range(nb_):
            tps = prj_tps.tile([P, P], BF16, tag="tps")
            nc.tensor.transpose(tps[:], ft[:, s, :], ident[:])
            tsb = prj_tsb.tile([P, P], BF16, tag="tsb")
            nc.any.tensor_copy(tsb[:], tps[:])
            ps = prj_ps.tile([P, 2 * c.feats], FP32, tag="kvps")
            nc.tensor.matmul(ps[:], tsb[:], wkv[:], start=True, stop=True)
            nc.any.tensor_copy(sb[:, s, :], ps[:])
        nc.sync.dma_start(
            kv_r[:, pair * PB : pair * PB + nb_, :], sb[:, 0:nb_, :]
        )

    def emit_q_pair(pair, prj):
        prj_ft, prj_tps, prj_tsb, prj_ps, prj_sb = prj
        blks = range(pair * PB, min((pair + 1) * PB, c.nblk))
        ft = prj_ft.tile([P, PB, P], BF16, tag="ft")
        nc.sync.dma_start(
            ft[:, 0 : len(blks), :],
            floc_r[:, pair * PB : pair * PB + len(blks), :],
        )
        for s, blk in enumerate(blks):
            tps = prj_tps.tile([P, P], BF16, tag="tps")
            nc.tensor.transpose(tps[:], ft[:, s, :], ident[:])
            tsb = prj_tsb.tile([P, P], BF16, tag="tsb")
            nc.any.tensor_copy(tsb[:], tps[:])
            ps = prj_ps.tile([P, 2 * c.feats], FP32, tag="kvps")
            nc.tensor.matmul(
                ps[:, 0 : c.feats], tsb[:], wq[:], start=True, stop=True
            )
            nc.any.tensor_copy(q_sb[:, blk, :], ps[:, 0 : c.feats])

    def guard_read(b0, b1, tag):
        # Tile's wait pass covers store DMA completions only transitively via
        # the sb-tile WAR reuse chain, which misses the last ~bufs stores.
        # Read back the tail with a regular load (RAW => real DMAHW wait) and
        # consume it on the gather engine so every gather orders after it.
        g_ = keep.tile([P, 16, 16], BF16, tag=tag)
        nc.sync.dma_start(g_[:, 0 : b1 - b0, :], kv_r[:, b0:b1, 0:16])
        nc.gpsimd.tensor_copy(g_[:, 0, :], g_[:, 1, :])

    def gather_rows(out_ap, in_ap, idx_tile, idx_off, n):
        nc.gpsimd.dma_gather(
            out_ap,
            in_ap,
            idx_tile[:, idx_off // 16 : (idx_off + n) // 16],
            n,
            n,
            2 * c.feats,
            elem_step=2 * c.feats,
            transpose=False,
            single_packet=False,
        )

    # ---------- phase 3 (interleaved): epilogue helpers ----------

    def layernorm(pool, x32, nb, out_dtype=FP32):
        """x32: [P, nb, 128] fp32 SBUF tile -> normalized * g + b (new tile)."""
        msum = pool.tile([P, EPB], FP32, tag="ln_msum")
        nc.vector.tensor_reduce(
            msum[:, 0:nb], x32[:, 0:nb, :], axis=mybir.AxisListType.X, op=OP.add
        )
        nmean = pool.tile([P, EPB], FP32, tag="ln_nmean")
        nc.vector.tensor_scalar_mul(nmean[:, 0:nb], msum[:, 0:nb], -1.0 / c.feats)
        sq = pool.tile([P, EPB, P], FP32, tag="ln_sq")
        for b in range(nb):
            nc.scalar.activation(
                sq[:, b],
                x32[:, b],
                AF.Square,
                bias=nmean[:, b : b + 1],
            )
        var = pool.tile([P, EPB], FP32, tag="ln_var")
        nc.vector.tensor_reduce(
            var[:, 0:nb], sq[:, 0:nb, :], axis=mybir.AxisListType.X, op=OP.add
        )
        rstd = pool.tile([P, EPB], FP32, tag="ln_rstd")
        nc.vector.tensor_scalar(
            rstd[:, 0:nb], var[:, 0:nb], 1.0 / c.feats, 1e-5, op0=OP.mult, op1=OP.add
        )
        nc.vector.reciprocal(rstd[:, 0:nb], rstd[:, 0:nb])
        nc.scalar.sqrt(rstd[:, 0:nb], rstd[:, 0:nb])
        nmr = pool.tile([P, EPB], FP32, tag="ln_nmr")
        nc.vector.tensor_tensor(
            nmr[:, 0:nb], nmean[:, 0:nb], rstd[:, 0:nb], op=OP.mult
        )
        normed = pool.tile([P, EPB, P], FP32, tag="ln_normed")
        for b in range(nb):
            nc.scalar.activation(
                normed[:, b],
                x32[:, b],
                AF.Identity,
                scale=rstd[:, b : b + 1],
                bias=nmr[:, b : b + 1],
            )
        out = pool.tile([P, EPB, P], out_dtype, tag="ln_out" + str(out_dtype))
        nc.vector.tensor_tensor(
            out[:, 0:nb],
            normed[:, 0:nb],
            grep[:].rearrange("p (o f) -> p o f", o=1).to_broadcast([P, nb, P]),
            op=OP.mult,
        )
        nc.vector.tensor_tensor(
            out[:, 0:nb],
            out[:, 0:nb],
            brep[:].rearrange("p (o f) -> p o f", o=1).to_broadcast([P, nb, P]),
            op=OP.add,
        )
        return out

    def epilogue(b0, nb):
        f32 = ep.tile([P, EPB, P], FP32, tag="f32")
        nc.sync.dma_start(
            f32[:, 0:nb, :],
            t["feat32_loc"][:]
            .rearrange("(s p) f -> p s f", p=P)[:, b0 : b0 + nb, :],
        )
        lo32 = ep.tile([P, EPB, 136], FP32, tag="lo32")
        nc.scalar.copy(lo32[:, 0:nb], ftden_lo[:, b0 : b0 + nb, :])
        tot = ep.tile([P, EPB, 136], FP32, tag="ftot")
        # eps folded into the add: guards the pad-slot denominators (num cols
        # shift by 1e-30 — far below bf16 noise)
        nc.vector.scalar_tensor_tensor(
            tot[:, 0:nb],
            ftden_sb[:, b0 : b0 + nb, :],
            1e-30,
            lo32[:, 0:nb],
            op0=OP.add,
            op1=OP.add,
        )
        r = ep.tile([P, EPB, c.heads], FP32, tag="recip")
        nc.vector.reciprocal(r[:, 0:nb], tot[:, 0:nb, 128:136])
        rst = ep.tile([P, EPB, P], FP32, tag="rst")
        nc.vector.tensor_tensor(
            rst[:, 0:nb],
            tot[:, 0:nb, 0:128].rearrange(
                "p s (h d) -> p s h d", d=c.dhead
            ),
            r[:, 0:nb].rearrange("p s (h o) -> p s h o", o=1).to_broadcast(
                [P, nb, c.heads, c.dhead]
            ),
            op=OP.mult,
        )
        nc.vector.tensor_tensor(
            rst[:, 0:nb], rst[:, 0:nb], f32[:, 0:nb, :], op=OP.add
        )
        ln1 = layernorm(ep, rst, nb)
        ln1b = ep.tile([P, EPB, P], BF16, tag="ln1b")
        nc.scalar.copy(ln1b[:, 0:nb], ln1[:, 0:nb])
        # transpose ln1 -> feat-major for FFN
        rT_ps = ep_ps.tile([P, EPB * P], BF16, tag="rT_ps")
        for b in range(nb):
            nc.tensor.transpose(
                rT_ps[:, b * P : (b + 1) * P], ln1b[:, b, :], ident[:]
            )
        rT = ep.tile([P, EPB * P], BF16, tag="rT")
        nc.vector.tensor_copy(rT[:, 0 : nb * P], rT_ps[:, 0 : nb * P])
        # H1 = W1.T @ rT  (feat-major, nh slices) ; prelu via W2a/W2b trick
        ffps = ep_ps.tile([P, EPB * P], FP32, tag="ffps")
        for h in range(nh):
            h1ps = ep_h1ps.tile([P, EPB * P], FP32, tag="h1ps")
            nc.tensor.matmul(
                h1ps[:, 0 : nb * P],
                w1[:, h * P : (h + 1) * P],
                rT[:, 0 : nb * P],
                start=True,
                stop=True,
            )
            h1sb = ep.tile([P, EPB * P], BF16, tag="h1sb")
            nc.scalar.activation(
                h1sb[:, 0 : nb * P],
                h1ps[:, 0 : nb * P],
                AF.Identity,
                bias=b1t[:, h : h + 1],
            )
            habs = ep.tile([P, EPB * P], BF16, tag="habs")
            nc.scalar.activation(
                habs[:, 0 : nb * P],
                h1ps[:, 0 : nb * P],
                AF.Abs,
                bias=b1t[:, h : h + 1],
            )
            for b in range(nb):
                nc.tensor.matmul(
                    ffps[:, b * P : (b + 1) * P],
                    h1sb[:, b * P : (b + 1) * P],
                    w2a[:, h, :],
                    start=(h == 0 and b == 0),
                    stop=False,
                    skip_group_check=True,
                )
                nc.tensor.matmul(
                    ffps[:, b * P : (b + 1) * P],
                    habs[:, b * P : (b + 1) * P],
                    w2b[:, h, :],
                    start=False,
                    stop=(h == nh - 1),
                    skip_group_check=True,
                )
        rst2 = ep.tile([P, EPB, P], FP32, tag="rst2")
        nc.vector.tensor_tensor(
            rst2[:, 0:nb],
            ffps[:, 0 : nb * P].rearrange("p (s f) -> p s f", f=P),
            ln1[:, 0:nb],
            op=OP.add,
        )
        nc.vector.tensor_tensor(
            rst2[:, 0:nb],
            rst2[:, 0:nb],
            b2rep[:].rearrange("p (o f) -> p o f", o=1).to_broadcast([P, nb, P]),
            op=OP.add,
        )
        ln2 = layernorm(ep, rst2, nb)
        nc.sync.dma_start(
            t["out"][:].rearrange("(s p) f -> p s f", p=P)[:, b0 : b0 + nb, :],
            ln2[:, 0:nb],
        )




    # ---------- phase 2: two-sweep edge processing ----------
    smax_h = max(max(g["L_lo"], g["L_hi"]) for g in groups) // P
    EPB = c.grp  # blocks per epilogue call (one gather group)

    with (
        tc.tile_pool(name="eg_kv", bufs=4) as eg_kv,
        tc.tile_pool(name="eg_s", bufs=2) as eg_s,
        tc.tile_pool(name="eg_tt", bufs=2) as eg_tt,
        tc.tile_pool(name="ep", bufs=2) as ep,
        tc.tile_pool(name="eg_qps", bufs=2, space="PSUM") as eg_qps,
        tc.tile_pool(name="eg_ftps", bufs=2, space="PSUM") as eg_ftps,
    ):

        def sweep_group(g, h, off):
            """Process one group's lo (h=0) or hi (h=1) edges; returns new off."""
            Lh = g["L_lo"] if h == 0 else g["L_hi"]
            colrel = 0 if h == 0 else g["L_lo"]
            base = g["scol"] + colrel
            ns = Lh // P
            kvE = eg_kv.tile([P, smax_h, 2 * c.feats], BF16, tag="kvE")
            Ssb = eg_s.tile([P, smax_h * P], BF16, tag="Ssb")
            STsb = eg_s.tile([P, smax_h * P], BF16, tag="STsb")
            nc.sync.dma_start(Ssb[:, 0:Lh], t["S_all"][:, base : base + Lh])
            nc.sync.dma_start(STsb[:, 0:Lh], t["ST_all"][:, base : base + Lh])
            gather_rows(
                kvE[:, 0:ns, :],
                kv_table[:][0 : c.half, :]
                if h == 0
                else kv_table[:][c.half : c.npad, :],
                kvlo if h == 0 else kvhi,
                off,
                Lh,
            )
            runs_h = [r for r in g["runs"] if r["half"] == h]
            # per-sub: qE select (edge-major) + per-edge k*q products
            # qE[e, f] = sum_d ST[d, e] * q_blk[d, f]
            TT = eg_tt.tile([P, smax_h, P], BF16, tag="TT")
            for r in runs_h:
                c0 = r["col"] - colrel
                for k in range(r["ncols"] // P):
                    si = c0 // P + k
                    qps = eg_qps.tile([P, P], FP32, tag="qps")
                    nc.tensor.matmul(
                        qps[:],
                        STsb[:, c0 + k * P : c0 + (k + 1) * P],
                        q_sb[:, r["block"], :],
                        start=True,
                        stop=True,
                    )
                    nc.vector.tensor_tensor(
                        TT[:, si, :], kvE[:, si, 0 : c.feats], qps[:], op=OP.mult
                    )
            # group-half-wide: per-head score reduce, exp, weighted messages
            scores = eg_tt.tile([P, smax_h * c.heads], FP32, tag="scores")
            nc.vector.tensor_reduce(
                scores[:, 0 : ns * c.heads],
                TT[:, 0:ns, :].rearrange("p a (h d) -> p (a h) d", d=c.dhead),
                axis=mybir.AxisListType.X,
                op=OP.add,
            )
            pexp = eg_tt.tile([P, smax_h * c.heads], BF16, tag="pexp")
            nc.scalar.activation(
                pexp[:, 0 : ns * c.heads],
                scores[:, 0 : ns * c.heads],
                AF.Exp,
                scale=scale,
            )
            Mt = eg_tt.tile([P, smax_h, P], BF16, tag="Mt")
            nc.vector.tensor_tensor(
                Mt[:, 0:ns].rearrange("p a (h d) -> p a h d", d=c.dhead),
                kvE[:, 0:ns, c.feats : 2 * c.feats].rearrange(
                    "p a (h d) -> p a h d", d=c.dhead
                ),
                pexp[:, 0 : ns * c.heads]
                .rearrange("p (a h o) -> p a h o", h=c.heads, o=1)
                .to_broadcast([P, ns, c.heads, c.dhead]),
                op=OP.mult,
            )
            for r in runs_h:
                b = r["block"]
                c0 = r["col"] - colrel
                nsr = r["ncols"] // P
                ftp = eg_ftps.tile([P, 136], FP32, tag="ftps", name="ftps")
                for k in range(nsr):
                    si = c0 // P + k
                    nc.tensor.matmul(
                        ftp[:, 0:128],
                        Ssb[:, c0 + k * P : c0 + (k + 1) * P],
                        Mt[:, si, :],
                        start=k == 0,
                        stop=k == nsr - 1,
                        skip_group_check=True,
                    )
                    # ft2's start already marked this bank pending-zero, so
                    # the first denom write lands on zeroed bytes (start=False)
                    nc.tensor.matmul(
                        ftp[:, 128:136],
                        Ssb[:, c0 + k * P : c0 + (k + 1) * P],
                        pexp[:, si * c.heads : (si + 1) * c.heads],
                        start=False,
                        stop=k == nsr - 1,
                        skip_group_check=True,
                    )
                nc.any.tensor_copy(
                    (ftden_lo if h == 0 else ftden_sb)[:, b, :], ftp[:]
                )
            if h == 1:
                epilogue(g["bs"][0], len(g["bs"]))
            return off + Lh

        # sweep A: build lo table, then lo gathers with the hi table build
        # interleaved (different engines/rows -> full overlap)
        with (
            tc.tile_pool(name="prj_ft", bufs=3) as prj_ft,
            tc.tile_pool(name="prj_tps", bufs=2, space="PSUM") as prj_tps,
            tc.tile_pool(name="prj_tsb", bufs=2) as prj_tsb,
            tc.tile_pool(name="prj_ps", bufs=2, space="PSUM") as prj_ps,
            tc.tile_pool(name="prj_sb", bufs=3) as prj_sb,
        ):
            prj = (prj_ft, prj_tps, prj_tsb, prj_ps, prj_sb)
            for pair in range(n_pairsA):
                emit_pair(pair, prj)
            for pair in range((c.nblk + PB - 1) // PB):
                emit_q_pair(pair, prj)
            guard_read(lo_blocks - 16, lo_blocks, "guard_lo")
            per = (len(pairsB) + len(groups) - 1) // len(groups)
            pbi = 0
            olo = 0
            for g in groups:
                for _ in range(per):
                    if pbi < len(pairsB):
                        emit_pair(pairsB[pbi], prj)
                        pbi += 1
                olo = sweep_group(g, 0, olo)
            while pbi < len(pairsB):
                emit_pair(pairsB[pbi], prj)
                pbi += 1
        # sweep B: hi gathers + scatter + interleaved epilogue
        guard_read(nbl - 16, nbl, "guard_hi")
        with (
            tc.tile_pool(name="ep_ps", bufs=1, space="PSUM") as ep_ps,
            tc.tile_pool(name="ep_h1ps", bufs=2, space="PSUM") as ep_h1ps,
        ):
            ohi = 0
            for g in groups:
                ohi = sweep_group(g, 1, ohi)

    if c.debug:
        nc.sync.dma_start(t["dbg_ftden"][:], ftden_sb[:])

def _build(meta, cfg: GATCfg):
    c = cfg
    nc = bacc.Bacc("TRN2", target_bir_lowering=False, debug=False, num_devices=c.n_cores)
    t = {}

    def inp(name, shape, dtype):
        t[name] = nc.dram_tensor(name, shape, dtype, kind="ExternalInput").ap()

    inp("feat16", [c.npad, c.feats], BF16)
    inp("feat16_loc", [c.local_pad, c.feats], BF16)
    inp("feat32_loc", [c.local_pad, c.feats], FP32)
    inp("wq", [c.feats, c.feats], BF16)
    inp("wkv", [c.feats, 2 * c.feats], BF16)
    inp("w1", [c.feats, c.dff], BF16)
    inp("w2a", [P, c.dff // P, c.feats], BF16)
    inp("w2b", [P, c.dff // P, c.feats], BF16)
    inp("b1t", [P, c.dff // P], FP32)
    inp("b2rep", [P, c.feats], FP32)
    inp("grep", [P, c.feats], FP32)
    inp("brep", [P, c.feats], FP32)
    inp("ident", [P, P], BF16)
    inp("kv_idx_lo", [P, max(meta["tot_lo"] // 16, 1)], I16)
    inp("kv_idx_hi", [P, max(meta["tot_hi"] // 16, 1)], I16)
    inp("S_all", [P, meta["tot_cols"]], BF16)
    inp("ST_all", [P, meta["tot_cols"]], BF16)
    t["out"] = nc.dram_tensor(
        "out", [c.local_pad, c.feats], FP32, kind="ExternalOutput"
    ).ap()
    if c.debug:
        t["dbg_ftden"] = nc.dram_tensor(
            "dbg_ftden", [P, c.nblk, 136], FP32, kind="ExternalOutput"
        ).ap()

    with tile.TileContext(nc) as tc:
        _emit(tc, t, meta, cfg)
    nc.compile()
    return nc


def _in_maps(meta, streams, shared, cfg: GATCfg):
    maps = []
    for ci in range(cfg.n_cores):
        m = dict(shared)
        st = streams[ci]
        feat32_loc = st["feat32_loc"]
        m["feat16_loc"] = feat32_loc.astype(bf16)
        m["feat32_loc"] = feat32_loc
        m["kv_idx_lo"] = (
            st["kv_idx_lo"]
            if meta["tot_lo"]
            else np.zeros((P, 1), np.int16)
        )
        m["kv_idx_hi"] = (
            st["kv_idx_hi"]
            if meta["tot_hi"]
            else np.zeros((P, 1), np.int16)
        )
        m["S_all"] = st["S_all"]
        m["ST_all"] = st["ST_all"]
        maps.append(m)
    return maps


_CACHE = {}


def kernel(**inputs) -> np.ndarray:
    cfg = GATCfg()
    meta, streams, shared = _prep(inputs, cfg)
    key = "real"
    if key not in _CACHE:
        _CACHE[key] = _build(meta, cfg)
    nc = _CACHE[key]
    maps = _in_maps(meta, streams, shared, cfg)
    res = run_bass_kernel_spmd(nc, maps, core_ids=list(range(cfg.n_cores)))
    out = np.empty((cfg.n_nodes, cfg.feats), np.float32)
    for ci in range(cfg.n_cores):
        out[ci * cfg.npc : (ci + 1) * cfg.npc] = res.results[ci]["out"][: cfg.npc]
    return out



# revision 7
# speedup vs baseline: 1.4700x; 1.4700x over previous
"""GAT message-passing layer on 8 Trainium2 NeuronCores (Bass/Tile).

v2 strategy (no SWDGE gather): nodes are partitioned across the 8 cores; each
edge is owned by the core that owns its destination node, so segment softmax
and the weighted scatter-sum stay core-local.  Instead of building a k/v
table in DRAM and gathering per-edge rows with SWDGE (descriptor-generation
serialized ~750us in v1), the HOST pre-duplicates the source-node features
into edge order (a purely structural permutation of the input, like the
one-hot S/ST matrices) and stores them feature-major.  The kernel streams
[featE | S | ST] with one large HWDGE DMA per group and computes the k/v
projections PER EDGE on the PE:

  per 128-edge sub:  kvE[e,256] = featE_sub^T @ [Wk|Wv]   (lhsT = featE_sub)
                     qE [e,128] = ST_sub^T @ q_blk        (one-hot select)
  per 2-sub chunk:   TT = qE_bf16 * kE   (DVE) ; scores = head-reduce (DVE)
                     pexp = exp(scores)  (Scalar, into Mt[:,:,128:136])
                     Mt[:,:,0:128] = vE * pexp            (DVE)
  per sub:           ftp[d,136] += S_sub^T @ Mt_sub       (PE scatter, num+den)

The epilogue (divide, residual, LN, FFN with PReLU folded into two weight
matrices, LN) runs per 2-block group right after its scatters finish.
"""

import sys

sys.path.insert(0, "/opt/trn_rl_repo")

import math
from contextlib import ExitStack
from dataclasses import dataclass

import numpy as np
import ml_dtypes

import concourse.bass as bass
import concourse.bacc as bacc
import concourse.mybir as mybir
import concourse.tile as tile
from concourse._compat import with_exitstack
from concourse.bass_utils import run_bass_kernel_spmd

bf16 = ml_dtypes.bfloat16
P = 128
AF = mybir.ActivationFunctionType
OP = mybir.AluOpType
FP32 = mybir.dt.float32
BF16 = mybir.dt.bfloat16


@dataclass
class GATCfg:
    n_nodes: int = 50000
    n_edges: int = 640000
    feats: int = 128
    heads: int = 8
    dhead: int = 16
    dff: int = 512
    n_cores: int = 8
    grp: int = 2  # dst blocks per group (epilogue batch)
    csz: int = 2  # subs per score/message chunk

    @property
    def npc(self):  # nodes per core
        return self.n_nodes // self.n_cores

    @property
    def nblk(self):  # local 128-node blocks per core
        return (self.npc + P - 1) // P

    @property
    def local_pad(self):
        return self.nblk * P

    @property
    def ngrp(self):
        return (self.nblk + self.grp - 1) // self.grp


def _prep(inputs, cfg: GATCfg):
    """Host-side graph partitioning, padding, stream assembly."""
    c = cfg
    feat = np.asarray(inputs["feat"], np.float32)
    src = np.asarray(inputs["src"], np.int64)
    dst = np.asarray(inputs["dst"], np.int64)

    # feature table, feature-major, with a zero column for pad edges
    featT = np.zeros((c.feats, c.n_nodes + 1), np.float32)
    featT[:, : c.n_nodes] = feat.T
    featT16 = featT.astype(bf16)

    # ---- per (core, block) edge lists ----
    core_of = dst // c.npc
    per_core = []
    for ci in range(c.n_cores):
        sel = np.nonzero(core_of == ci)[0]
        dloc = dst[sel] - ci * c.npc
        blk = dloc // P
        order = np.lexsort((dloc, blk))
        sel, dloc, blk = sel[order], dloc[order], blk[order]
        lists = {}
        for b in range(c.nblk):
            m = blk == b
            lists[b] = (src[sel[m]], dloc[m])
        per_core.append(lists)

    # uniform sub counts across cores (SPMD single program)
    ns = np.zeros(c.nblk, np.int64)
    for b in range(c.nblk):
        mx = max(len(per_core[ci][b][0]) for ci in range(c.n_cores))
        ns[b] = max((mx + P - 1) // P, 1)

    # ---- group structure (shared across cores) ----
    groups = []
    scol = 0
    for g in range(c.ngrp):
        bs = list(range(g * c.grp, min((g + 1) * c.grp, c.nblk)))
        base = []  # per-block column base within the group
        off = 0
        for b in bs:
            base.append(off)
            off += int(ns[b]) * P
        groups.append(dict(bs=bs, base=base, L=off, scol=scol))
        scol += off
    tot_cols = scol
    Lmax = max(g["L"] for g in groups)

    meta = dict(groups=groups, tot_cols=tot_cols, Lmax=Lmax, ns=ns)

    # ---- per-core streams: SALL = per-group [featE | S | ST] ----
    per_core_streams = []
    for ci in range(c.n_cores):
        src_idx = np.full(tot_cols, c.n_nodes, np.int64)  # pad -> zero col
        S = np.zeros((P, tot_cols), np.float32)
        ST = np.zeros((P, tot_cols), np.float32)
        for g in groups:
            for b, b0 in zip(g["bs"], g["base"]):
                s_arr, d_arr = per_core[ci][b]
                col0 = g["scol"] + b0
                n = len(s_arr)
                pos = np.arange(n)
                src_idx[col0 : col0 + n] = s_arr
                ss = pos // P
                pp = pos % P
                dslot = d_arr - b * P
                S[pp, col0 + ss * P + dslot] = 1.0
                ST[dslot, col0 + pos] = 1.0
        featE = featT16[:, src_idx]  # [128, tot_cols] bf16
        S16 = S.astype(bf16)
        ST16 = ST.astype(bf16)
        SALL = np.empty((P, 3 * tot_cols), bf16)
        for g in groups:
            s0, L = g["scol"], g["L"]
            SALL[:, 3 * s0 : 3 * s0 + L] = featE[:, s0 : s0 + L]
            SALL[:, 3 * s0 + L : 3 * s0 + 2 * L] = S16[:, s0 : s0 + L]
            SALL[:, 3 * s0 + 2 * L : 3 * s0 + 3 * L] = ST16[:, s0 : s0 + L]

        feat32_loc = np.zeros((c.local_pad, c.feats), np.float32)
        feat32_loc[: c.npc] = feat[ci * c.npc : (ci + 1) * c.npc]
        featlocT = np.zeros((c.feats, c.local_pad), np.float32)
        featlocT[:, : c.npc] = feat[ci * c.npc : (ci + 1) * c.npc].T
        per_core_streams.append(
            dict(
                SALL=SALL,
                feat32_loc=feat32_loc,
                feat16_locT=featlocT.astype(bf16),
            )
        )

    # ---- shared weight/constant tensors ----
    W1 = np.asarray(inputs["W1"], np.float32)
    W2 = np.asarray(inputs["W2"], np.float32)
    a = np.asarray(inputs["prelu_a"], np.float32)
    # prelu(x) = max(x,0) + a*min(x,0) = ((1+a)/2)*x + ((1-a)/2)*|x|
    nh = c.dff // P
    W2a = (
        (((1.0 + a) / 2.0)[:, None] * W2)
        .reshape(nh, P, c.feats)
        .transpose(1, 0, 2)
        .astype(bf16)
    )
    W2b = (
        (((1.0 - a) / 2.0)[:, None] * W2)
        .reshape(nh, P, c.feats)
        .transpose(1, 0, 2)
        .astype(bf16)
    )
    scale = 1.0 / math.sqrt(c.heads * c.dhead)
    wkv = np.concatenate(
        [np.asarray(inputs["Wk"], np.float32), np.asarray(inputs["Wv"], np.float32)],
        axis=1,
    )
    shared = dict(
        wq=(np.asarray(inputs["Wq"], np.float32) * scale).astype(bf16),
        wkv=wkv.astype(bf16),
        w1=W1.astype(bf16),
        w2a=W2a,
        w2b=W2b,
        b1t=np.ascontiguousarray(
            np.asarray(inputs["b1"], np.float32).reshape(nh, P).T
        ),
        b2rep=np.tile(np.asarray(inputs["b2"], np.float32)[None, :], (P, 1)),
        grep=np.tile(np.asarray(inputs["ln1_g"], np.float32)[None, :], (P, 1)),
        brep=np.tile(np.asarray(inputs["ln1_b"], np.float32)[None, :], (P, 1)),
        ident=np.eye(P, dtype=np.float32).astype(bf16),
    )
    return meta, per_core_streams, shared


@with_exitstack
def _emit(ctx: ExitStack, tc: tile.TileContext, t, meta, cfg: GATCfg):
    c = cfg
    nc = tc.nc
    groups = meta["groups"]
    ns = meta["ns"]
    Lmax = meta["Lmax"]
    nh = c.dff // P
    EPB = c.grp
    NSG = Lmax // P  # max subs per group

    # ---------- persistent pool: constants + per-block q ----------
    keep = ctx.enter_context(tc.tile_pool(name="keep", bufs=1))

    def load_const(name, shape, dtype):
        tl = keep.tile(shape, dtype, tag=name)
        nc.sync.dma_start(tl[:], t[name][:])
        return tl

    wq = load_const("wq", [P, P], BF16)
    wkv = load_const("wkv", [P, 2 * c.feats], BF16)
    w1 = load_const("w1", [P, c.dff], BF16)
    w2a = load_const("w2a", [P, nh, c.feats], BF16)
    w2b = load_const("w2b", [P, nh, c.feats], BF16)
    b1t = load_const("b1t", [P, nh], FP32)
    b2rep = load_const("b2rep", [P, P], FP32)
    grep = load_const("grep", [P, P], FP32)
    brep = load_const("brep", [P, P], FP32)
    ident = load_const("ident", [P, P], BF16)
    flocT = load_const("feat16_locT", [P, c.local_pad], BF16)

    q_sb = keep.tile([P, c.nblk, c.feats], BF16, tag="q_sb")

    # ---------- epilogue helpers (shared tiles declared in pools below) ----

    def layernorm(pool, x32, nb, out_dtype=FP32):
        msum = pool.tile([P, EPB], FP32, tag="ln_msum")
        nc.vector.tensor_reduce(
            msum[:, 0:nb], x32[:, 0:nb, :], axis=mybir.AxisListType.X, op=OP.add
        )
        nmean = pool.tile([P, EPB], FP32, tag="ln_nmean")
        nc.vector.tensor_scalar_mul(nmean[:, 0:nb], msum[:, 0:nb], -1.0 / c.feats)
        sq = pool.tile([P, EPB, P], FP32, tag="ln_sq")
        for b in range(nb):
            nc.scalar.activation(
                sq[:, b], x32[:, b], AF.Square, bias=nmean[:, b : b + 1]
            )
        var = pool.tile([P, EPB], FP32, tag="ln_var")
        nc.vector.tensor_reduce(
            var[:, 0:nb], sq[:, 0:nb, :], axis=mybir.AxisListType.X, op=OP.add
        )
        rstd = pool.tile([P, EPB], FP32, tag="ln_rstd")
        nc.vector.tensor_scalar(
            rstd[:, 0:nb], var[:, 0:nb], 1.0 / c.feats, 1e-5, op0=OP.mult, op1=OP.add
        )
        nc.vector.reciprocal(rstd[:, 0:nb], rstd[:, 0:nb])
        nc.scalar.sqrt(rstd[:, 0:nb], rstd[:, 0:nb])
        nmr = pool.tile([P, EPB], FP32, tag="ln_nmr")
        nc.vector.tensor_tensor(
            nmr[:, 0:nb], nmean[:, 0:nb], rstd[:, 0:nb], op=OP.mult
        )
        normed = pool.tile([P, EPB, P], FP32, tag="ln_normed")
        for b in range(nb):
            nc.scalar.activation(
                normed[:, b],
                x32[:, b],
                AF.Identity,
                scale=rstd[:, b : b + 1],
                bias=nmr[:, b : b + 1],
            )
        out = pool.tile([P, EPB, P], out_dtype, tag="ln_out" + str(out_dtype))
        nc.vector.tensor_tensor(
            out[:, 0:nb],
            normed[:, 0:nb],
            grep[:].rearrange("p (o f) -> p o f", o=1).to_broadcast([P, nb, P]),
            op=OP.mult,
        )
        nc.vector.tensor_tensor(
            out[:, 0:nb],
            out[:, 0:nb],
            brep[:].rearrange("p (o f) -> p o f", o=1).to_broadcast([P, nb, P]),
            op=OP.add,
        )
        return out

    # ---------- main pools ----------
    with (
        tc.tile_pool(name="gt", bufs=2) as gt_pool,
        tc.tile_pool(name="mt", bufs=2) as mt_pool,
        tc.tile_pool(name="sc", bufs=2) as sc_pool,
        tc.tile_pool(name="ep", bufs=2) as ep,
        tc.tile_pool(name="kvps", bufs=2, space="PSUM") as kv_pool,
        tc.tile_pool(name="qps", bufs=1, space="PSUM") as q_pool,
        tc.tile_pool(name="ftps", bufs=2, space="PSUM") as ft_pool,
        tc.tile_pool(name="ep_ps", bufs=1, space="PSUM") as ep_ps,
        tc.tile_pool(name="ep_h1ps", bufs=1, space="PSUM") as ep_h1ps,
    ):
        # ---- per-block q projection ----
        for b in range(c.nblk):
            qp = q_pool.tile([P, c.csz, P], FP32, tag="qps")
            nc.tensor.matmul(
                qp[:, 0, :],
                flocT[:, b * P : (b + 1) * P],
                wq[:],
                start=True,
                stop=True,
            )
            nc.scalar.copy(q_sb[:, b, :], qp[:, 0, :])

        def epilogue(g, ftps, f32):
            bs = g["bs"]
            nb = len(bs)
            tot = ep.tile([P, EPB, 136], FP32, tag="ftot")
            for bi in range(nb):
                # eps guards pad-slot denominators
                nc.vector.tensor_scalar(
                    tot[:, bi], ftps[bi], 1.0, 1e-30, op0=OP.mult, op1=OP.add
                )
            r = ep.tile([P, EPB, c.heads], FP32, tag="recip")
            nc.vector.reciprocal(r[:, 0:nb], tot[:, 0:nb, 128:136])
            rst = ep.tile([P, EPB, P], FP32, tag="rst")
            nc.vector.tensor_tensor(
                rst[:, 0:nb],
                tot[:, 0:nb, 0:128].rearrange("p s (h d) -> p s h d", d=c.dhead),
                r[:, 0:nb]
                .rearrange("p s (h o) -> p s h o", o=1)
                .to_broadcast([P, nb, c.heads, c.dhead]),
                op=OP.mult,
            )
            nc.vector.tensor_tensor(
                rst[:, 0:nb], rst[:, 0:nb], f32[:, 0:nb, :], op=OP.add
            )
            ln1 = layernorm(ep, rst, nb)
            ln1b = ep.tile([P, EPB, P], BF16, tag="ln1b")
            nc.scalar.copy(ln1b[:, 0:nb], ln1[:, 0:nb])
            rT_ps = ep_ps.tile([P, EPB * P], BF16, tag="rT_ps")
            for b in range(nb):
                nc.tensor.transpose(
                    rT_ps[:, b * P : (b + 1) * P], ln1b[:, b, :], ident[:]
                )
            rT = ep.tile([P, EPB * P], BF16, tag="rT")
            nc.vector.tensor_copy(rT[:, 0 : nb * P], rT_ps[:, 0 : nb * P])
            ffps = ep_ps.tile([P, EPB * P], FP32, tag="ffps")
            for h in range(nh):
                h1ps = ep_h1ps.tile([P, EPB * P], FP32, tag="h1ps")
                nc.tensor.matmul(
                    h1ps[:, 0 : nb * P],
                    w1[:, h * P : (h + 1) * P],
                    rT[:, 0 : nb * P],
                    start=True,
                    stop=True,
                )
                h1sb = ep.tile([P, EPB * P], BF16, tag="h1sb")
                nc.scalar.activation(
                    h1sb[:, 0 : nb * P],
                    h1ps[:, 0 : nb * P],
                    AF.Identity,
                    bias=b1t[:, h : h + 1],
                )
                habs = ep.tile([P, EPB * P], BF16, tag="habs")
                nc.scalar.activation(
                    habs[:, 0 : nb * P],
                    h1ps[:, 0 : nb * P],
                    AF.Abs,
                    bias=b1t[:, h : h + 1],
                )
                for b in range(nb):
                    nc.tensor.matmul(
                        ffps[:, b * P : (b + 1) * P],
                        h1sb[:, b * P : (b + 1) * P],
                        w2a[:, h, :],
                        start=(h == 0 and b == 0),
                        stop=False,
                        skip_group_check=True,
                    )
                    nc.tensor.matmul(
                        ffps[:, b * P : (b + 1) * P],
                        habs[:, b * P : (b + 1) * P],
                        w2b[:, h, :],
                        start=False,
                        stop=(h == nh - 1),
                        skip_group_check=True,
                    )
            rst2 = ep.tile([P, EPB, P], FP32, tag="rst2")
            nc.vector.tensor_tensor(
                rst2[:, 0:nb],
                ffps[:, 0 : nb * P].rearrange("p (s f) -> p s f", f=P),
                ln1[:, 0:nb],
                op=OP.add,
            )
            nc.vector.tensor_tensor(
                rst2[:, 0:nb],
                rst2[:, 0:nb],
                b2rep[:].rearrange("p (o f) -> p o f", o=1).to_broadcast([P, nb, P]),
                op=OP.add,
            )
            ln2 = layernorm(ep, rst2, nb)
            nc.sync.dma_start(
                t["out"][:].rearrange("(s p) f -> p s f", p=P)[
                    :, bs[0] : bs[0] + nb, :
                ],
                ln2[:, 0:nb],
            )

        # ---- main loop over groups ----
        for g in groups:
            L = g["L"]
            s0 = g["scol"]
            gt = gt_pool.tile([P, 3 * Lmax], BF16, tag="gt")
            nc.sync.dma_start(gt[:, 0 : 3 * L], t["SALL"][:, 3 * s0 : 3 * s0 + 3 * L])
            f32 = ep.tile([P, EPB, P], FP32, tag="f32")
            nc.sync.dma_start(
                f32[:, 0 : len(g["bs"]), :],
                t["feat32_loc"][:]
                .rearrange("(s p) f -> p s f", p=P)[
                    :, g["bs"][0] : g["bs"][0] + len(g["bs"]), :
                ],
            )
            Mt = mt_pool.tile([P, NSG, 136], BF16, tag="Mt")
            ftg = ft_pool.tile([P, EPB, 136], FP32, tag="ftp")
            ftps = []
            for bi, (b, b0) in enumerate(zip(g["bs"], g["base"])):
                nsb = int(ns[b])
                ftp = ftg[:, bi, :]
                for c0 in range(0, nsb, c.csz):
                    cs = min(c.csz, nsb - c0)
                    mtb = (b0 // P) + c0  # sub index within group (Mt row)
                    kvps = kv_pool.tile([P, c.csz, 2, P], FP32, tag="kvps")
                    qps = q_pool.tile([P, c.csz, P], FP32, tag="qps")
                    for s in range(cs):
                        col = b0 + (c0 + s) * P
                        nc.tensor.matmul(
                            kvps[:, s].rearrange("p t f -> p (t f)"),
                            gt[:, col : col + P],
                            wkv[:],
                            start=True,
                            stop=True,
                        )
                        nc.tensor.matmul(
                            qps[:, s],
                            gt[:, 2 * L + col : 2 * L + col + P],
                            q_sb[:, b, :],
                            start=True,
                            stop=True,
                        )
                    qcp = sc_pool.tile([P, c.csz, P], BF16, tag="qcp")
                    nc.scalar.copy(qcp[:, 0:cs], qps[:, 0:cs])
                    tt = sc_pool.tile([P, c.csz, P], BF16, tag="tt")
                    nc.vector.tensor_tensor(
                        tt[:, 0:cs], qcp[:, 0:cs], kvps[:, 0:cs, 0, :], op=OP.mult
                    )
                    sc = sc_pool.tile([P, c.csz, c.heads], FP32, tag="sc")
                    nc.vector.tensor_reduce(
                        sc[:, 0:cs].rearrange("p s h -> p (s h)"),
                        tt[:, 0:cs].rearrange("p s (h d) -> p (s h) d", d=c.dhead),
                        axis=mybir.AxisListType.X,
                        op=OP.add,
                    )
                    nc.scalar.activation(
                        Mt[:, mtb : mtb + cs, 128:136],
                        sc[:, 0:cs],
                        AF.Exp,
                    )
                    nc.vector.tensor_tensor(
                        Mt[:, mtb : mtb + cs, 0:128].rearrange(
                            "p s (h d) -> p s h d", d=c.dhead
                        ),
                        kvps[:, 0:cs, 1, :].rearrange(
                            "p s (h d) -> p s h d", d=c.dhead
                        ),
                        Mt[:, mtb : mtb + cs, 128:136]
                        .rearrange("p s (h o) -> p s h o", o=1)
                        .to_broadcast([P, cs, c.heads, c.dhead]),
                        op=OP.mult,
                    )
                    for s in range(cs):
                        col = b0 + (c0 + s) * P
                        nc.tensor.matmul(
                            ftp,
                            gt[:, L + col : L + col + P],
                            Mt[:, mtb + s, :],
                            start=(c0 + s == 0),
                            stop=(c0 + s == nsb - 1),
                            skip_group_check=True,
                        )
                ftps.append(ftp)
            epilogue(g, ftps, f32)


def _build(meta, cfg: GATCfg):
    c = cfg
    nc = bacc.Bacc(
        "TRN2", target_bir_lowering=False, debug=False, num_devices=c.n_cores
    )
    t = {}

    def inp(name, shape, dtype):
        t[name] = nc.dram_tensor(name, shape, dtype, kind="ExternalInput").ap()

    inp("SALL", [P, 3 * meta["tot_cols"]], BF16)
    inp("feat16_locT", [P, c.local_pad], BF16)
    inp("feat32_loc", [c.local_pad, c.feats], FP32)
    inp("wq", [c.feats, c.feats], BF16)
    inp("wkv", [c.feats, 2 * c.feats], BF16)
    inp("w1", [c.feats, c.dff], BF16)
    inp("w2a", [P, c.dff // P, c.feats], BF16)
    inp("w2b", [P, c.dff // P, c.feats], BF16)
    inp("b1t", [P, c.dff // P], FP32)
    inp("b2rep", [P, c.feats], FP32)
    inp("grep", [P, c.feats], FP32)
    inp("brep", [P, c.feats], FP32)
    inp("ident", [P, P], BF16)
    t["out"] = nc.dram_tensor(
        "out", [c.local_pad, c.feats], FP32, kind="ExternalOutput"
    ).ap()

    with tile.TileContext(nc) as tc:
        _emit(tc, t, meta, cfg)
    nc.compile()
    return nc


def _in_maps(meta, streams, shared, cfg: GATCfg):
    maps = []
    for ci in range(cfg.n_cores):
        m = dict(shared)
        m.update(streams[ci])
        maps.append(m)
    return maps


_CACHE = {}


def kernel(**inputs) -> np.ndarray:
    cfg = GATCfg()
    meta, streams, shared = _prep(inputs, cfg)
    key = "real"
    if key not in _CACHE:
        _CACHE[key] = _build(meta, cfg)
    nc = _CACHE[key]
    maps = _in_maps(meta, streams, shared, cfg)
    res = run_bass_kernel_spmd(nc, maps, core_ids=list(range(cfg.n_cores)))
    out = np.empty((cfg.n_nodes, cfg.feats), np.float32)
    for ci in range(cfg.n_cores):
        out[ci * cfg.npc : (ci + 1) * cfg.npc] = res.results[ci]["out"][: cfg.npc]
    return out


# revision 12
# speedup vs baseline: 1.6448x; 1.1189x over previous
"""GAT message-passing layer on 8 Trainium2 NeuronCores (Bass/Tile).

v4: nodes partitioned across 8 cores; edges owned by their dst core so the
segment softmax and scatter-sum stay local.  The HOST pre-duplicates
source-node features into edge order feature-major (structural permutation,
like the one-hot S/ST matrices), so the kernel streams [featE | S | ST] with
one HWDGE DMA per group — no SWDGE row gather.

Per 512-edge chunk (f-major score path), emitted as a 3-stage software
pipeline A(c) / B1(c-1) / B2(c-2) so no engine ever waits on a same-chunk
cross-engine dependency:

  A:  kE' [f,e]  = Wk^T @ featE_chunk       (PE, N=512)
      qE' [f,e]  = q_blk^T-select via ST    (PE, N=512, lhsT=q_blk)
      qcp        = bf16(qE')                (Scalar copy, PSUM->SBUF)
      TT  [f,e]  = kE' * qcp                (DVE)
  B1: scores[e,8]= TT_sub^T @ Hsel          (PE, start=False onto memset-0)
      pexp       = exp(scores)              (Scalar -> Mt[:,:,128:136])
      vE  [e,f]  = featE_sub^T @ Wv         (PE)
      Mt[:,:,0:128] = vE * pexp             (DVE)
  B2: ftp [d,136] += S_sub^T @ Mt_sub       (PE scatter, num+denominator)

PSUM discipline: `start=True` clears the whole bank's has_written bits, so
any matmul sharing a bank with an open accumulation uses start=False onto
DVE-memset bytes (scores, FFN h1/ffps).  Epilogue per 2-block group:
divide, residual, LN, FFN (native Prelu), LN.  LN's rsqrt is computed on
the DVE (two-segment linear seed + 3 Newton steps) so the scalar engine
only ever uses {exp, square, identity, copy, parametric_relu} — all in the
first activation-table set; zero ACT_TABLE_LOAD thrash.
"""

import sys

sys.path.insert(0, "/opt/trn_rl_repo")

import math
from contextlib import ExitStack
from dataclasses import dataclass

import numpy as np
import ml_dtypes

import concourse.bass as bass
import concourse.bacc as bacc
import concourse.mybir as mybir
import concourse.tile as tile
from concourse._compat import with_exitstack
from concourse.bass_utils import run_bass_kernel_spmd

bf16 = ml_dtypes.bfloat16
P = 128
AF = mybir.ActivationFunctionType
OP = mybir.AluOpType
FP32 = mybir.dt.float32
BF16 = mybir.dt.bfloat16

# two-segment linear seed for Newton rsqrt (fit on var' in [0.25, 9])
RSA1, RSB1 = 1.73846, 0.54441
RSA2, RSB2 = 0.74615, 0.04950


@dataclass
class GATCfg:
    n_nodes: int = 50000
    n_edges: int = 640000
    feats: int = 128
    heads: int = 8
    dhead: int = 16
    dff: int = 512
    n_cores: int = 8
    grp: int = 2  # dst blocks per group (epilogue batch)
    csz: int = 4  # subs per chunk

    @property
    def npc(self):
        return self.n_nodes // self.n_cores

    @property
    def nblk(self):
        return (self.npc + P - 1) // P

    @property
    def local_pad(self):
        return self.nblk * P

    @property
    def ngrp(self):
        return (self.nblk + self.grp - 1) // self.grp


def _prep(inputs, cfg: GATCfg):
    """Host-side graph partitioning, padding, stream assembly."""
    c = cfg
    feat = np.asarray(inputs["feat"], np.float32)
    src = np.asarray(inputs["src"], np.int64)
    dst = np.asarray(inputs["dst"], np.int64)

    featT = np.zeros((c.feats, c.n_nodes + 1), np.float32)
    featT[:, : c.n_nodes] = feat.T
    featT16 = featT.astype(bf16)

    core_of = dst // c.npc
    per_core = []
    for ci in range(c.n_cores):
        sel = np.nonzero(core_of == ci)[0]
        dloc = dst[sel] - ci * c.npc
        blk = dloc // P
        order = np.lexsort((dloc, blk))
        sel, dloc, blk = sel[order], dloc[order], blk[order]
        lists = {}
        for b in range(c.nblk):
            m = blk == b
            lists[b] = (src[sel[m]], dloc[m])
        per_core.append(lists)

    ns = np.zeros(c.nblk, np.int64)
    for b in range(c.nblk):
        mx = max(len(per_core[ci][b][0]) for ci in range(c.n_cores))
        ns[b] = max((mx + P - 1) // P, 1)

    groups = []
    scol = 0
    for g in range(c.ngrp):
        bs = list(range(g * c.grp, min((g + 1) * c.grp, c.nblk)))
        base = []
        off = 0
        for b in bs:
            base.append(off)
            off += int(ns[b]) * P
        groups.append(dict(bs=bs, base=base, L=off, scol=scol, gi=g))
        scol += off
    tot_cols = scol
    Lmax = max(g["L"] for g in groups)

    meta = dict(groups=groups, tot_cols=tot_cols, Lmax=Lmax, ns=ns)

    per_core_streams = []
    for ci in range(c.n_cores):
        src_idx = np.full(tot_cols, c.n_nodes, np.int64)  # pad -> zero col
        S = np.zeros((P, tot_cols), np.float32)
        ST = np.zeros((P, tot_cols), np.float32)
        for g in groups:
            for b, b0 in zip(g["bs"], g["base"]):
                s_arr, d_arr = per_core[ci][b]
                col0 = g["scol"] + b0
                n = len(s_arr)
                pos = np.arange(n)
                src_idx[col0 : col0 + n] = s_arr
                dslot = d_arr - b * P
                S[pos % P, col0 + (pos // P) * P + dslot] = 1.0
                ST[dslot, col0 + pos] = 1.0
        featE = featT16[:, src_idx]
        S16 = S.astype(bf16)
        ST16 = ST.astype(bf16)
        SALL = np.empty((P, 3 * tot_cols), bf16)
        for g in groups:
            s0, L = g["scol"], g["L"]
            SALL[:, 3 * s0 : 3 * s0 + L] = featE[:, s0 : s0 + L]
            SALL[:, 3 * s0 + L : 3 * s0 + 2 * L] = S16[:, s0 : s0 + L]
            SALL[:, 3 * s0 + 2 * L : 3 * s0 + 3 * L] = ST16[:, s0 : s0 + L]

        feat32_loc = np.zeros((c.local_pad, c.feats), np.float32)
        feat32_loc[: c.npc] = feat[ci * c.npc : (ci + 1) * c.npc]
        featlocT = np.zeros((c.feats, c.local_pad), np.float32)
        featlocT[:, : c.npc] = feat[ci * c.npc : (ci + 1) * c.npc].T
        per_core_streams.append(
            dict(
                SALL=SALL,
                feat32_loc=feat32_loc,
                feat16_locT=featlocT.astype(bf16),
            )
        )

    W1 = np.asarray(inputs["W1"], np.float32)
    W2 = np.asarray(inputs["W2"], np.float32)
    a = np.asarray(inputs["prelu_a"], np.float32)
    nh = c.dff // P
    W2t = W2.reshape(nh, P, c.feats).transpose(1, 0, 2).astype(bf16)
    scale = 1.0 / math.sqrt(c.heads * c.dhead)
    hsel = np.zeros((P, c.heads), np.float32)
    hsel[np.arange(P), np.arange(P) // c.dhead] = 1.0
    shared = dict(
        wq=(np.asarray(inputs["Wq"], np.float32) * scale).astype(bf16),
        wk=np.asarray(inputs["Wk"], np.float32).astype(bf16),
        wv=np.asarray(inputs["Wv"], np.float32).astype(bf16),
        w1=W1.astype(bf16),
        w2=W2t,
        b1t=np.ascontiguousarray(
            np.asarray(inputs["b1"], np.float32).reshape(nh, P).T
        ),
        at=np.ascontiguousarray(a.reshape(nh, P).T),
        b2rep=np.tile(np.asarray(inputs["b2"], np.float32)[None, :], (P, 1)),
        grep=np.tile(np.asarray(inputs["ln1_g"], np.float32)[None, :], (P, 1)),
        brep=np.tile(np.asarray(inputs["ln1_b"], np.float32)[None, :], (P, 1)),
        ident=np.eye(P, dtype=np.float32).astype(bf16),
        hsel=hsel.astype(bf16),
    )
    return meta, per_core_streams, shared


@with_exitstack
def _emit(ctx: ExitStack, tc: tile.TileContext, t, meta, cfg: GATCfg):
    c = cfg
    nc = tc.nc
    groups = meta["groups"]
    ns = meta["ns"]
    Lmax = meta["Lmax"]
    nh = c.dff // P
    EPB = c.grp
    NSG = Lmax // P  # max subs per group

    keep = ctx.enter_context(tc.tile_pool(name="keep", bufs=1))

    def load_const(name, shape, dtype):
        tl = keep.tile(shape, dtype, tag=name)
        nc.sync.dma_start(tl[:], t[name][:])
        return tl

    wq = load_const("wq", [P, P], BF16)
    wk = load_const("wk", [P, P], BF16)
    wv = load_const("wv", [P, P], BF16)
    w1 = load_const("w1", [P, c.dff], BF16)
    w2 = load_const("w2", [P, nh, c.feats], BF16)
    b1t = load_const("b1t", [P, nh], FP32)
    at = load_const("at", [P, nh], FP32)
    b2rep = load_const("b2rep", [P, P], FP32)
    grep = load_const("grep", [P, P], FP32)
    brep = load_const("brep", [P, P], FP32)
    ident = load_const("ident", [P, P], BF16)
    hsel = load_const("hsel", [P, c.heads], BF16)
    flocT = load_const("feat16_locT", [P, c.local_pad], BF16)

    q_sb = keep.tile([P, c.nblk, c.feats], BF16, tag="q_sb")

    # misc PSUM bank layout (fp32 cols): ftp_b0 [0:136), ftp_b1 [136:272),
    # score slots [272:336) (2 x 32, chunk parity), FFN rT [352:480) as bf16
    SC0 = 272
    RT0 = 352
    # ffh1 bank layout: ffps [0:EPB*P), h1ps [EPB*P : 2*EPB*P)
    H10 = EPB * P

    with (
        tc.tile_pool(name="gt", bufs=2) as gt_pool,
        tc.tile_pool(name="qcp", bufs=2) as qcp_pool,
        tc.tile_pool(name="tt", bufs=2) as tt_pool,
        tc.tile_pool(name="mt", bufs=2) as mt_pool,
        tc.tile_pool(name="ep", bufs=2) as ep,
        tc.tile_pool(name="kps", bufs=2, space="PSUM") as k_pool,
        tc.tile_pool(name="qps", bufs=2, space="PSUM") as q_pool,
        tc.tile_pool(name="vps", bufs=1, space="PSUM") as v_pool,
        tc.tile_pool(name="misc", bufs=2, space="PSUM") as misc_pool,
        tc.tile_pool(name="ffh1", bufs=1, space="PSUM") as ff_pool,
    ):
        # ---- per-block q projection (node-major q_blk [d, f]) ----
        for b in range(c.nblk):
            qp = q_pool.tile([P, c.csz * P], FP32, tag="qps")
            nc.tensor.matmul(
                qp[:, 0:P],
                flocT[:, b * P : (b + 1) * P],
                wq[:],
                start=True,
                stop=True,
            )
            nc.scalar.copy(q_sb[:, b, :], qp[:, 0:P])

        def newton_rsqrt(pool, vq, nb):
            """rstd = 1/sqrt(vq), DVE-only (no scalar act table use)."""
            s1 = pool.tile([P, EPB], FP32, tag="rs_s1")
            s2 = pool.tile([P, EPB], FP32, tag="rs_s2")
            y = pool.tile([P, EPB], FP32, tag="rs_y")
            u = pool.tile([P, EPB], FP32, tag="rs_u")
            nc.vector.tensor_scalar(
                s1[:, 0:nb], vq[:, 0:nb], -RSB1, RSA1, op0=OP.mult, op1=OP.add
            )
            nc.vector.tensor_scalar(
                s2[:, 0:nb], vq[:, 0:nb], -RSB2, RSA2, op0=OP.mult, op1=OP.add
            )
            nc.vector.tensor_tensor(y[:, 0:nb], s1[:, 0:nb], s2[:, 0:nb], op=OP.max)
            for _ in range(3):
                nc.vector.tensor_tensor(
                    u[:, 0:nb], vq[:, 0:nb], y[:, 0:nb], op=OP.mult
                )
                nc.vector.tensor_tensor(
                    u[:, 0:nb], u[:, 0:nb], y[:, 0:nb], op=OP.mult
                )
                nc.vector.tensor_scalar(
                    u[:, 0:nb], u[:, 0:nb], -0.5, 1.5, op0=OP.mult, op1=OP.add
                )
                nc.vector.tensor_tensor(
                    y[:, 0:nb], y[:, 0:nb], u[:, 0:nb], op=OP.mult
                )
            return y

        def layernorm(pool, x32, nb, out_dtype=FP32):
            msum = pool.tile([P, EPB], FP32, tag="ln_msum")
            nc.vector.tensor_reduce(
                msum[:, 0:nb], x32[:, 0:nb, :], axis=mybir.AxisListType.X, op=OP.add
            )
            nmean = pool.tile([P, EPB], FP32, tag="ln_nmean")
            nc.vector.tensor_scalar_mul(
                nmean[:, 0:nb], msum[:, 0:nb], -1.0 / c.feats
            )
            sq = pool.tile([P, EPB, P], FP32, tag="ln_sq")
            var = pool.tile([P, EPB], FP32, tag="ln_var")
            for b in range(nb):
                nc.scalar.activation(
                    sq[:, b],
                    x32[:, b],
                    AF.Square,
                    bias=nmean[:, b : b + 1],
                    accum_out=var[:, b : b + 1],
                )
            vq = pool.tile([P, EPB], FP32, tag="ln_vq")
            nc.vector.tensor_scalar(
                vq[:, 0:nb], var[:, 0:nb], 1.0 / c.feats, 1e-5, op0=OP.mult, op1=OP.add
            )
            rstd = newton_rsqrt(pool, vq, nb)
            nmr = pool.tile([P, EPB], FP32, tag="ln_nmr")
            nc.vector.tensor_tensor(
                nmr[:, 0:nb], nmean[:, 0:nb], rstd[:, 0:nb], op=OP.mult
            )
            normed = pool.tile([P, EPB, P], FP32, tag="ln_normed")
            for b in range(nb):
                nc.scalar.activation(
                    normed[:, b],
                    x32[:, b],
                    AF.Identity,
                    scale=rstd[:, b : b + 1],
                    bias=nmr[:, b : b + 1],
                )
            out = pool.tile([P, EPB, P], out_dtype, tag="ln_out" + str(out_dtype))
            nc.vector.tensor_tensor(
                out[:, 0:nb],
                normed[:, 0:nb],
                grep[:].rearrange("p (o f) -> p o f", o=1).to_broadcast([P, nb, P]),
                op=OP.mult,
            )
            nc.vector.tensor_tensor(
                out[:, 0:nb],
                out[:, 0:nb],
                brep[:].rearrange("p (o f) -> p o f", o=1).to_broadcast([P, nb, P]),
                op=OP.add,
            )
            return out

        def epilogue(g):
            bs = g["bs"]
            nb = len(bs)
            misc = g["misc"]
            f32 = g["f32"]
            tot = ep.tile([P, EPB, 136], FP32, tag="ftot")
            for bi in range(nb):
                nc.vector.tensor_scalar(
                    tot[:, bi],
                    misc[:, bi * 136 : bi * 136 + 136],
                    1.0,
                    1e-30,
                    op0=OP.mult,
                    op1=OP.add,
                )
            r = ep.tile([P, EPB, c.heads], FP32, tag="recip")
            nc.vector.reciprocal(r[:, 0:nb], tot[:, 0:nb, 128:136])
            rst = ep.tile([P, EPB, P], FP32, tag="rst")
            nc.vector.tensor_tensor(
                rst[:, 0:nb],
                tot[:, 0:nb, 0:128].rearrange("p s (h d) -> p s h d", d=c.dhead),
                r[:, 0:nb]
                .rearrange("p s (h o) -> p s h o", o=1)
                .to_broadcast([P, nb, c.heads, c.dhead]),
                op=OP.mult,
            )
            nc.vector.tensor_tensor(
                rst[:, 0:nb], rst[:, 0:nb], f32[:, 0:nb, :], op=OP.add
            )
            ln1 = layernorm(ep, rst, nb, out_dtype=BF16)
            for b in range(nb):
                nc.tensor.transpose(
                    misc[:, RT0 + b * 64 : RT0 + (b + 1) * 64].bitcast(BF16),
                    ln1[:, b, :],
                    ident[:],
                )
            rT = ep.tile([P, EPB * P], BF16, tag="rT")
            nc.vector.tensor_copy(
                rT[:, 0 : nb * P],
                misc[:, RT0 : RT0 + nb * 64].bitcast(BF16),
            )
            ffh1 = ff_pool.tile([P, 2 * EPB * P], FP32, tag="ffh1")
            nc.vector.memset(ffh1[:, 0 : nb * P], 0.0)
            for h in range(nh):
                nc.vector.memset(ffh1[:, H10 : H10 + nb * P], 0.0)
                nc.tensor.matmul(
                    ffh1[:, H10 : H10 + nb * P],
                    w1[:, h * P : (h + 1) * P],
                    rT[:, 0 : nb * P],
                    start=False,
                    stop=True,
                    skip_group_check=True,
                )
                h1p = ep.tile([P, EPB * P], BF16, tag="h1p")
                nc.scalar.activation(
                    h1p[:, 0 : nb * P],
                    ffh1[:, H10 : H10 + nb * P],
                    AF.Prelu,
                    bias=b1t[:, h : h + 1],
                    alpha=at[:, h : h + 1],
                )
                for b in range(nb):
                    nc.tensor.matmul(
                        ffh1[:, b * P : (b + 1) * P],
                        h1p[:, b * P : (b + 1) * P],
                        w2[:, h, :],
                        start=False,
                        stop=(h == nh - 1),
                        skip_group_check=True,
                    )
            rst2 = ep.tile([P, EPB, P], FP32, tag="rst2")
            nc.vector.tensor_tensor(
                rst2[:, 0:nb],
                ffh1[:, 0 : nb * P].rearrange("p (s f) -> p s f", f=P),
                ln1[:, 0:nb],
                op=OP.add,
            )
            nc.vector.tensor_tensor(
                rst2[:, 0:nb],
                rst2[:, 0:nb],
                b2rep[:].rearrange("p (o f) -> p o f", o=1).to_broadcast([P, nb, P]),
                op=OP.add,
            )
            ln2 = layernorm(ep, rst2, nb)
            nc.sync.dma_start(
                t["out"][:].rearrange("(s p) f -> p s f", p=P)[
                    :, bs[0] : bs[0] + nb, :
                ],
                ln2[:, 0:nb],
            )

        # ---- chunk list over all groups/blocks ----
        chunks = []
        for g in groups:
            for bi, (b, b0) in enumerate(zip(g["bs"], g["base"])):
                nsb = int(ns[b])
                for ci in range(0, nsb, c.csz):
                    cs = min(c.csz, nsb - ci)
                    chunks.append(
                        dict(
                            g=g, bi=bi, b=b, b0=b0, ci=ci, cs=cs,
                            mtb=b0 // P + ci, nsb=nsb,
                            last_of_group=False,
                        )
                    )
            chunks[-1]["last_of_group"] = True

        def group_setup(g):
            L = g["L"]
            s0 = g["scol"]
            gt = gt_pool.tile([P, 3 * Lmax], BF16, tag="gt")
            nc.sync.dma_start(
                gt[:, 0 : 3 * L], t["SALL"][:, 3 * s0 : 3 * s0 + 3 * L]
            )
            f32 = ep.tile([P, EPB, P], FP32, tag="f32")
            nc.sync.dma_start(
                f32[:, 0 : len(g["bs"]), :],
                t["feat32_loc"][:]
                .rearrange("(s p) f -> p s f", p=P)[
                    :, g["bs"][0] : g["bs"][0] + len(g["bs"]), :
                ],
            )
            g["gt"] = gt
            g["f32"] = f32
            g["Mt"] = mt_pool.tile([P, NSG, 136], BF16, tag="Mt", name="Mt")
            g["misc"] = misc_pool.tile([P, 512], FP32, tag="misc", name="misc")

        def stage_a(ch, idx):
            g = ch["g"]
            gt, L, b0, ci, cs = g["gt"], g["L"], ch["b0"], ch["ci"], ch["cs"]
            kps = k_pool.tile([P, c.csz * P], FP32, tag="kps")
            nc.tensor.matmul(
                kps[:, 0 : cs * P],
                wk[:],
                gt[:, b0 + ci * P : b0 + (ci + cs) * P],
                start=True,
                stop=True,
            )
            qps = q_pool.tile([P, c.csz * P], FP32, tag="qps")
            nc.tensor.matmul(
                qps[:, 0 : cs * P],
                q_sb[:, ch["b"], :],
                gt[:, 2 * L + b0 + ci * P : 2 * L + b0 + (ci + cs) * P],
                start=True,
                stop=True,
            )
            qcp = qcp_pool.tile([P, c.csz * P], BF16, tag="qcp")
            nc.scalar.copy(qcp[:, 0 : cs * P], qps[:, 0 : cs * P])
            tt = tt_pool.tile([P, c.csz, P], BF16, tag="tt")
            nc.vector.tensor_tensor(
                tt[:, 0:cs],
                qcp[:, 0 : cs * P].rearrange("p (s f) -> p s f", f=P),
                kps[:, 0 : cs * P].rearrange("p (s f) -> p s f", f=P),
                op=OP.mult,
            )
            ch["tt"] = tt

        def stage_b1(ch, idx):
            g = ch["g"]
            gt, L, b0, ci, cs, mtb = (
                g["gt"], g["L"], ch["b0"], ch["ci"], ch["cs"], ch["mtb"],
            )
            Mt, misc, tt = g["Mt"], g["misc"], ch["tt"]
            soff = SC0 + (idx % 2) * 32
            nc.vector.memset(misc[:, soff : soff + cs * c.heads], 0.0)
            for s in range(cs):
                nc.tensor.matmul(
                    misc[:, soff + s * c.heads : soff + (s + 1) * c.heads],
                    tt[:, s, :],
                    hsel[:],
                    start=False,
                    stop=True,
                    skip_group_check=True,
                )
            nc.scalar.activation(
                Mt[:, mtb : mtb + cs, 128:136],
                misc[:, soff : soff + cs * c.heads].rearrange(
                    "p (s h) -> p s h", h=c.heads
                ),
                AF.Exp,
            )
            vps = v_pool.tile([P, c.csz, P], FP32, tag="vps")
            for s in range(cs):
                col = b0 + (ci + s) * P
                nc.tensor.matmul(
                    vps[:, s], gt[:, col : col + P], wv[:], start=True, stop=True
                )
            nc.vector.tensor_tensor(
                Mt[:, mtb : mtb + cs, 0:128].rearrange(
                    "p s (h d) -> p s h d", d=c.dhead
                ),
                vps[:, 0:cs].rearrange("p s (h d) -> p s h d", d=c.dhead),
                Mt[:, mtb : mtb + cs, 128:136]
                .rearrange("p s (h o) -> p s h o", o=1)
                .to_broadcast([P, cs, c.heads, c.dhead]),
                op=OP.mult,
            )

        def stage_b2(ch, idx):
            g = ch["g"]
            gt, L, b0, ci, cs, mtb = (
                g["gt"], g["L"], ch["b0"], ch["ci"], ch["cs"], ch["mtb"],
            )
            Mt, misc = g["Mt"], g["misc"]
            for s in range(cs):
                col = b0 + (ci + s) * P
                nc.tensor.matmul(
                    misc[:, ch["bi"] * 136 : ch["bi"] * 136 + 136],
                    gt[:, L + col : L + col + P],
                    Mt[:, mtb + s, :],
                    start=(ci + s == 0),
                    stop=(ci + s == ch["nsb"] - 1),
                    skip_group_check=True,
                )

        # ---- software-pipelined emission ----
        n = len(chunks)
        cur_g = None
        for i in range(n + 2):
            if i < n:
                ch = chunks[i]
                if ch["g"] is not cur_g:
                    cur_g = ch["g"]
                    group_setup(cur_g)
                stage_a(ch, i)
            if 1 <= i <= n:
                stage_b1(chunks[i - 1], i - 1)
            if 2 <= i <= n + 1:
                ch2 = chunks[i - 2]
                stage_b2(ch2, i - 2)
                if ch2["last_of_group"]:
                    epilogue(ch2["g"])


def _build(meta, cfg: GATCfg):
    c = cfg
    nc = bacc.Bacc(
        "TRN2", target_bir_lowering=False, debug=False, num_devices=c.n_cores
    )
    t = {}

    def inp(name, shape, dtype):
        t[name] = nc.dram_tensor(name, shape, dtype, kind="ExternalInput").ap()

    inp("SALL", [P, 3 * meta["tot_cols"]], BF16)
    inp("feat16_locT", [P, c.local_pad], BF16)
    inp("feat32_loc", [c.local_pad, c.feats], FP32)
    inp("wq", [c.feats, c.feats], BF16)
    inp("wk", [c.feats, c.feats], BF16)
    inp("wv", [c.feats, c.feats], BF16)
    inp("w1", [c.feats, c.dff], BF16)
    inp("w2", [P, c.dff // P, c.feats], BF16)
    inp("b1t", [P, c.dff // P], FP32)
    inp("at", [P, c.dff // P], FP32)
    inp("b2rep", [P, c.feats], FP32)
    inp("grep", [P, c.feats], FP32)
    inp("brep", [P, c.feats], FP32)
    inp("ident", [P, P], BF16)
    inp("hsel", [P, c.heads], BF16)
    t["out"] = nc.dram_tensor(
        "out", [c.local_pad, c.feats], FP32, kind="ExternalOutput"
    ).ap()

    with tile.TileContext(nc) as tc:
        _emit(tc, t, meta, cfg)
    nc.compile()
    return nc


def _in_maps(meta, streams, shared, cfg: GATCfg):
    maps = []
    for ci in range(cfg.n_cores):
        m = dict(shared)
        m.update(streams[ci])
        maps.append(m)
    return maps


_CACHE = {}


def kernel(**inputs) -> np.ndarray:
    cfg = GATCfg()
    meta, streams, shared = _prep(inputs, cfg)
    key = "real"
    if key not in _CACHE:
        _CACHE[key] = _build(meta, cfg)
    nc = _CACHE[key]
    maps = _in_maps(meta, streams, shared, cfg)
    res = run_bass_kernel_spmd(nc, maps, core_ids=list(range(cfg.n_cores)))
    out = np.empty((cfg.n_nodes, cfg.feats), np.float32)
    for ci in range(cfg.n_cores):
        out[ci * cfg.npc : (ci + 1) * cfg.npc] = res.results[ci]["out"][: cfg.npc]
    return out


# revision 16
# speedup vs baseline: 1.9656x; 1.1950x over previous
"""GAT message-passing layer on 8 Trainium2 NeuronCores (Bass/Tile).

v4: nodes partitioned across 8 cores; edges owned by their dst core so the
segment softmax and scatter-sum stay local.  The HOST pre-duplicates
source-node features into edge order feature-major (structural permutation,
like the one-hot S/ST matrices), so the kernel streams [featE | S | ST] with
one HWDGE DMA per group — no SWDGE row gather.

Per 512-edge chunk (f-major score path), emitted as a 3-stage software
pipeline A(c) / B1(c-1) / B2(c-2) so no engine ever waits on a same-chunk
cross-engine dependency:

  A:  kE' [f,e]  = Wk^T @ featE_chunk       (PE, N=512)
      qE' [f,e]  = q_blk^T-select via ST    (PE, N=512, lhsT=q_blk)
      qcp        = bf16(qE')                (Scalar copy, PSUM->SBUF)
      TT  [f,e]  = kE' * qcp                (DVE)
  B1: scores[e,8]= TT_sub^T @ Hsel          (PE, start=False onto memset-0)
      pexp       = exp(scores)              (Scalar -> Mt[:,:,128:136])
      vE  [e,f]  = featE_sub^T @ Wv         (PE)
      Mt[:,:,0:128] = vE * pexp             (DVE)
  B2: ftp [d,136] += S_sub^T @ Mt_sub       (PE scatter, num+denominator)

PSUM discipline: `start=True` clears the whole bank's has_written bits, so
any matmul sharing a bank with an open accumulation uses start=False onto
DVE-memset bytes (scores, FFN h1/ffps).  Epilogue per 2-block group:
divide, residual, LN, FFN (native Prelu), LN.  LN's rsqrt is computed on
the DVE (two-segment linear seed + 3 Newton steps) so the scalar engine
only ever uses {exp, square, identity, copy, parametric_relu} — all in the
first activation-table set; zero ACT_TABLE_LOAD thrash.
"""

import sys

sys.path.insert(0, "/opt/trn_rl_repo")

import math
from contextlib import ExitStack
from dataclasses import dataclass

import numpy as np
import ml_dtypes

import concourse.bass as bass
import concourse.bacc as bacc
import concourse.mybir as mybir
import concourse.tile as tile
from concourse._compat import with_exitstack
from concourse.bass_utils import run_bass_kernel_spmd

bf16 = ml_dtypes.bfloat16
P = 128
AF = mybir.ActivationFunctionType
OP = mybir.AluOpType
FP32 = mybir.dt.float32
BF16 = mybir.dt.bfloat16

# two-segment linear seed for Newton rsqrt (fit on var' in [0.25, 9])
RSA1, RSB1 = 1.73846, 0.54441
RSA2, RSB2 = 0.74615, 0.04950


@dataclass
class GATCfg:
    n_nodes: int = 50000
    n_edges: int = 640000
    feats: int = 128
    heads: int = 8
    dhead: int = 16
    dff: int = 512
    n_cores: int = 8
    grp: int = 2  # dst blocks per group (epilogue batch)
    csz: int = 4  # subs per chunk

    @property
    def npc(self):
        return self.n_nodes // self.n_cores

    @property
    def nblk(self):
        return (self.npc + P - 1) // P

    @property
    def local_pad(self):
        return self.nblk * P

    @property
    def ngrp(self):
        return (self.nblk + self.grp - 1) // self.grp


def _prep(inputs, cfg: GATCfg):
    """Host-side graph partitioning, padding, stream assembly."""
    c = cfg
    feat = np.asarray(inputs["feat"], np.float32)
    src = np.asarray(inputs["src"], np.int64)
    dst = np.asarray(inputs["dst"], np.int64)

    featT = np.zeros((c.feats, c.n_nodes + 1), np.float32)
    featT[:, : c.n_nodes] = feat.T
    featT16 = featT.astype(bf16)

    core_of = dst // c.npc
    per_core = []
    for ci in range(c.n_cores):
        sel = np.nonzero(core_of == ci)[0]
        dloc = dst[sel] - ci * c.npc
        blk = dloc // P
        order = np.lexsort((dloc, blk))
        sel, dloc, blk = sel[order], dloc[order], blk[order]
        lists = {}
        for b in range(c.nblk):
            m = blk == b
            lists[b] = (src[sel[m]], dloc[m])
        per_core.append(lists)

    ns = np.zeros(c.nblk, np.int64)
    for b in range(c.nblk):
        mx = max(len(per_core[ci][b][0]) for ci in range(c.n_cores))
        ns[b] = max((mx + P - 1) // P, 1)

    groups = []
    scol = 0
    for g in range(c.ngrp):
        bs = list(range(g * c.grp, min((g + 1) * c.grp, c.nblk)))
        base = []
        off = 0
        for b in bs:
            base.append(off)
            off += int(ns[b]) * P
        groups.append(dict(bs=bs, base=base, L=off, scol=scol, gi=g))
        scol += off
    tot_cols = scol
    Lmax = max(g["L"] for g in groups)

    meta = dict(groups=groups, tot_cols=tot_cols, Lmax=Lmax, ns=ns)

    per_core_streams = []
    for ci in range(c.n_cores):
        src_idx = np.full(tot_cols, c.n_nodes, np.int64)  # pad -> zero col
        S = np.zeros((P, tot_cols), np.float32)
        ST = np.zeros((P, tot_cols), np.float32)
        for g in groups:
            for b, b0 in zip(g["bs"], g["base"]):
                s_arr, d_arr = per_core[ci][b]
                col0 = g["scol"] + b0
                n = len(s_arr)
                pos = np.arange(n)
                src_idx[col0 : col0 + n] = s_arr
                dslot = d_arr - b * P
                S[pos % P, col0 + (pos // P) * P + dslot] = 1.0
                ST[dslot, col0 + pos] = 1.0
        featE = featT16[:, src_idx]
        S16 = S.astype(bf16)
        ST16 = ST.astype(bf16)
        SALL = np.empty((P, 3 * tot_cols), bf16)
        for g in groups:
            s0, L = g["scol"], g["L"]
            SALL[:, 3 * s0 : 3 * s0 + L] = featE[:, s0 : s0 + L]
            SALL[:, 3 * s0 + L : 3 * s0 + 2 * L] = S16[:, s0 : s0 + L]
            SALL[:, 3 * s0 + 2 * L : 3 * s0 + 3 * L] = ST16[:, s0 : s0 + L]

        feat32_loc = np.zeros((c.local_pad, c.feats), np.float32)
        feat32_loc[: c.npc] = feat[ci * c.npc : (ci + 1) * c.npc]
        featlocT = np.zeros((c.feats, c.local_pad), np.float32)
        featlocT[:, : c.npc] = feat[ci * c.npc : (ci + 1) * c.npc].T
        per_core_streams.append(
            dict(
                SALL=SALL,
                feat32_loc=feat32_loc,
                feat16_locT=featlocT.astype(bf16),
            )
        )

    W1 = np.asarray(inputs["W1"], np.float32)
    W2 = np.asarray(inputs["W2"], np.float32)
    a = np.asarray(inputs["prelu_a"], np.float32)
    nh = c.dff // P
    W2t = W2.reshape(nh, P, c.feats).transpose(1, 0, 2).astype(bf16)
    scale = 1.0 / math.sqrt(c.heads * c.dhead)
    hsel = np.zeros((P, c.heads), np.float32)
    hsel[np.arange(P), np.arange(P) // c.dhead] = 1.0
    shared = dict(
        wq=(np.asarray(inputs["Wq"], np.float32) * scale).astype(bf16),
        wk=np.asarray(inputs["Wk"], np.float32).astype(bf16),
        wv=np.asarray(inputs["Wv"], np.float32).astype(bf16),
        w1=W1.astype(bf16),
        w2=W2t,
        b1t=np.ascontiguousarray(
            np.asarray(inputs["b1"], np.float32).reshape(nh, P).T
        ),
        at=np.ascontiguousarray(a.reshape(nh, P).T),
        b2rep=np.tile(np.asarray(inputs["b2"], np.float32)[None, :], (P, 1)),
        grep=np.tile(np.asarray(inputs["ln1_g"], np.float32)[None, :], (P, 1)),
        brep=np.tile(np.asarray(inputs["ln1_b"], np.float32)[None, :], (P, 1)),
        ident=np.eye(P, dtype=np.float32).astype(bf16),
        hsel=hsel.astype(bf16),
    )
    meta["skip_gb"] = bool(
        np.all(np.asarray(inputs["ln1_g"]) == 1.0)
        and np.all(np.asarray(inputs["ln1_b"]) == 0.0)
    )
    meta["skip_b2"] = bool(np.all(np.asarray(inputs["b2"]) == 0.0))
    return meta, per_core_streams, shared


@with_exitstack
def _emit(ctx: ExitStack, tc: tile.TileContext, t, meta, cfg: GATCfg):
    c = cfg
    nc = tc.nc
    groups = meta["groups"]
    ns = meta["ns"]
    Lmax = meta["Lmax"]
    nh = c.dff // P
    EPB = c.grp
    NSG = Lmax // P  # max subs per group

    keep = ctx.enter_context(tc.tile_pool(name="keep", bufs=1))

    def load_const(name, shape, dtype):
        tl = keep.tile(shape, dtype, tag=name)
        nc.sync.dma_start(tl[:], t[name][:])
        return tl

    wq = load_const("wq", [P, P], BF16)
    wk = load_const("wk", [P, P], BF16)
    wv = load_const("wv", [P, P], BF16)
    w1 = load_const("w1", [P, c.dff], BF16)
    w2 = load_const("w2", [P, nh, c.feats], BF16)
    b1t = load_const("b1t", [P, nh], FP32)
    at = load_const("at", [P, nh], FP32)
    b2rep = load_const("b2rep", [P, P], FP32)
    grep = load_const("grep", [P, P], FP32)
    brep = load_const("brep", [P, P], FP32)
    ident = load_const("ident", [P, P], BF16)
    hsel = load_const("hsel", [P, c.heads], BF16)
    flocT = load_const("feat16_locT", [P, c.local_pad], BF16)

    q_sb = keep.tile([P, c.nblk, c.feats], BF16, tag="q_sb")

    # misc PSUM bank layout (fp32 cols): ftp_b0 [0:136), ftp_b1 [136:272),
    # score slots [272:336) (2 x 32, chunk parity), FFN rT [352:480) as bf16
    SC0 = 272
    RT0 = 352
    # ffh1 bank layout: ffps [0:EPB*P), h1ps [EPB*P : 2*EPB*P)
    H10 = EPB * P

    with (
        tc.tile_pool(name="gt", bufs=2) as gt_pool,
        tc.tile_pool(name="qcp", bufs=2) as qcp_pool,
        tc.tile_pool(name="tt", bufs=2) as tt_pool,
        tc.tile_pool(name="mt", bufs=2) as mt_pool,
        tc.tile_pool(name="ep", bufs=2) as ep,
        tc.tile_pool(name="kps", bufs=2, space="PSUM") as k_pool,
        tc.tile_pool(name="qps", bufs=2, space="PSUM") as q_pool,
        tc.tile_pool(name="vps", bufs=1, space="PSUM") as v_pool,
        tc.tile_pool(name="misc", bufs=2, space="PSUM") as misc_pool,
        tc.tile_pool(name="ffh1", bufs=1, space="PSUM") as ff_pool,
    ):
        # ---- per-block q projection (node-major q_blk [d, f]) ----
        for b in range(c.nblk):
            qp = q_pool.tile([P, c.csz * P], FP32, tag="qps")
            nc.tensor.matmul(
                qp[:, 0:P],
                flocT[:, b * P : (b + 1) * P],
                wq[:],
                start=True,
                stop=True,
            )
            nc.scalar.copy(q_sb[:, b, :], qp[:, 0:P])

        skip_gb = meta["skip_gb"]
        skip_b2 = meta["skip_b2"]

        def ln_stats(pool, x32, nb):
            """Mean/var/rstd/nmr.  The serial rsqrt Newton chain runs on the
            (otherwise idle) GpSimd engine so it never head-of-line blocks
            the DVE queue."""
            msum = pool.tile([P, EPB], FP32, tag="ln_msum")
            nc.vector.tensor_reduce(
                msum[:, 0:nb], x32[:, 0:nb, :], axis=mybir.AxisListType.X, op=OP.add
            )
            nmean = pool.tile([P, EPB], FP32, tag="ln_nmean")
            nc.vector.tensor_scalar_mul(
                nmean[:, 0:nb], msum[:, 0:nb], -1.0 / c.feats
            )
            sq = pool.tile([P, EPB, P], FP32, tag="ln_sq")
            var = pool.tile([P, EPB], FP32, tag="ln_var")
            for b in range(nb):
                nc.scalar.activation(
                    sq[:, b],
                    x32[:, b],
                    AF.Square,
                    bias=nmean[:, b : b + 1],
                    accum_out=var[:, b : b + 1],
                )
            vq = pool.tile([P, EPB], FP32, tag="ln_vq")
            nc.vector.tensor_scalar(
                vq[:, 0:nb], var[:, 0:nb], 1.0 / c.feats, 1e-5, op0=OP.mult, op1=OP.add
            )
            s1 = pool.tile([P, EPB], FP32, tag="rs_s1")
            s2 = pool.tile([P, EPB], FP32, tag="rs_s2")
            y = pool.tile([P, EPB], FP32, tag="rs_y")
            u = pool.tile([P, EPB], FP32, tag="rs_u")
            nc.vector.tensor_scalar(
                s1[:, 0:nb], vq[:, 0:nb], -RSB1, RSA1, op0=OP.mult, op1=OP.add
            )
            nc.vector.tensor_scalar(
                s2[:, 0:nb], vq[:, 0:nb], -RSB2, RSA2, op0=OP.mult, op1=OP.add
            )
            nc.vector.tensor_tensor(y[:, 0:nb], s1[:, 0:nb], s2[:, 0:nb], op=OP.max)
            for _ in range(3):
                # y <- y * (1.5 - 0.5 * vq * y^2), 3 fused stt ops per step
                nc.vector.scalar_tensor_tensor(
                    u[:, 0:nb], y[:, 0:nb], 1.0, y[:, 0:nb],
                    op0=OP.mult, op1=OP.mult,
                )
                nc.vector.scalar_tensor_tensor(
                    u[:, 0:nb], u[:, 0:nb], -0.5, vq[:, 0:nb],
                    op0=OP.mult, op1=OP.mult,
                )
                nc.vector.scalar_tensor_tensor(
                    y[:, 0:nb], u[:, 0:nb], 1.5, y[:, 0:nb],
                    op0=OP.add, op1=OP.mult,
                )
            nmr = pool.tile([P, EPB], FP32, tag="ln_nmr")
            nc.vector.tensor_tensor(
                nmr[:, 0:nb], nmean[:, 0:nb], y[:, 0:nb], op=OP.mult
            )
            return y, nmr

        def ln_apply(pool, x32, nb, rstd, nmr, out_dtype):
            if skip_gb:
                out = pool.tile(
                    [P, EPB, P], out_dtype, tag="ln_out" + str(out_dtype)
                )
                for b in range(nb):
                    nc.scalar.activation(
                        out[:, b],
                        x32[:, b],
                        AF.Identity,
                        scale=rstd[:, b : b + 1],
                        bias=nmr[:, b : b + 1],
                    )
                return out
            normed = pool.tile([P, EPB, P], FP32, tag="ln_normed")
            for b in range(nb):
                nc.scalar.activation(
                    normed[:, b],
                    x32[:, b],
                    AF.Identity,
                    scale=rstd[:, b : b + 1],
                    bias=nmr[:, b : b + 1],
                )
            out = pool.tile([P, EPB, P], out_dtype, tag="ln_out" + str(out_dtype))
            nc.vector.tensor_tensor(
                out[:, 0:nb],
                normed[:, 0:nb],
                grep[:].rearrange("p (o f) -> p o f", o=1).to_broadcast([P, nb, P]),
                op=OP.mult,
            )
            nc.vector.tensor_tensor(
                out[:, 0:nb],
                out[:, 0:nb],
                brep[:].rearrange("p (o f) -> p o f", o=1).to_broadcast([P, nb, P]),
                op=OP.add,
            )
            return out

        def epilogue_stages(g):
            """Split the per-group epilogue into stages, interleaved with the
            next group's sweep so serial chains don't stall engine queues."""
            bs = g["bs"]
            nb = len(bs)
            misc = g["misc"]
            st = {}

            def s0():
                tot = ep.tile([P, EPB, 136], FP32, tag="ftot")
                for bi in range(nb):
                    nc.vector.tensor_scalar(
                        tot[:, bi],
                        misc[:, bi * 136 : bi * 136 + 136],
                        1.0,
                        1e-30,
                        op0=OP.mult,
                        op1=OP.add,
                    )
                r = ep.tile([P, EPB, c.heads], FP32, tag="recip")
                nc.vector.reciprocal(r[:, 0:nb], tot[:, 0:nb, 128:136])
                rst = ep.tile([P, EPB, P], FP32, tag="rst")
                nc.vector.tensor_tensor(
                    rst[:, 0:nb],
                    tot[:, 0:nb, 0:128].rearrange(
                        "p s (h d) -> p s h d", d=c.dhead
                    ),
                    r[:, 0:nb]
                    .rearrange("p s (h o) -> p s h o", o=1)
                    .to_broadcast([P, nb, c.heads, c.dhead]),
                    op=OP.mult,
                )
                nc.vector.tensor_tensor(
                    rst[:, 0:nb], rst[:, 0:nb], g["f32"][:, 0:nb, :], op=OP.add
                )
                st["rst"] = rst

            def s1():
                st["r1"] = ln_stats(ep, st["rst"], nb)

            def s2():
                ln1 = ln_apply(ep, st["rst"], nb, *st["r1"], BF16)
                st["ln1"] = ln1
                for b in range(nb):
                    nc.tensor.transpose(
                        misc[:, RT0 + b * 64 : RT0 + (b + 1) * 64].bitcast(BF16),
                        ln1[:, b, :],
                        ident[:],
                    )
                rT = ep.tile([P, EPB * P], BF16, tag="rT")
                nc.vector.tensor_copy(
                    rT[:, 0 : nb * P],
                    misc[:, RT0 : RT0 + nb * 64].bitcast(BF16),
                )
                st["rT"] = rT
                ffh1 = ff_pool.tile([P, 2 * EPB * P], FP32, tag="ffh1")
                nc.vector.memset(ffh1[:, 0 : nb * P], 0.0)
                st["ffh1"] = ffh1

            def mk_h(h):
                def s_h():
                    ffh1 = st["ffh1"]
                    nc.vector.memset(ffh1[:, H10 : H10 + nb * P], 0.0)
                    nc.tensor.matmul(
                        ffh1[:, H10 : H10 + nb * P],
                        w1[:, h * P : (h + 1) * P],
                        st["rT"][:, 0 : nb * P],
                        start=False,
                        stop=True,
                        skip_group_check=True,
                    )
                    h1p = ep.tile([P, EPB * P], BF16, tag="h1p")
                    nc.scalar.activation(
                        h1p[:, 0 : nb * P],
                        ffh1[:, H10 : H10 + nb * P],
                        AF.Prelu,
                        bias=b1t[:, h : h + 1],
                        alpha=at[:, h : h + 1],
                    )
                    for b in range(nb):
                        nc.tensor.matmul(
                            ffh1[:, b * P : (b + 1) * P],
                            h1p[:, b * P : (b + 1) * P],
                            w2[:, h, :],
                            start=False,
                            stop=(h == nh - 1),
                            skip_group_check=True,
                        )
                return s_h

            def s5():
                rst2 = ep.tile([P, EPB, P], FP32, tag="rst2")
                nc.vector.tensor_tensor(
                    rst2[:, 0:nb],
                    st["ffh1"][:, 0 : nb * P].rearrange("p (s f) -> p s f", f=P),
                    st["ln1"][:, 0:nb],
                    op=OP.add,
                )
                if not skip_b2:
                    nc.vector.tensor_tensor(
                        rst2[:, 0:nb],
                        rst2[:, 0:nb],
                        b2rep[:]
                        .rearrange("p (o f) -> p o f", o=1)
                        .to_broadcast([P, nb, P]),
                        op=OP.add,
                    )
                st["rst2"] = rst2
                st["r2"] = ln_stats(ep, rst2, nb)

            def s6():
                ln2 = ln_apply(ep, st["rst2"], nb, *st["r2"], FP32)
                nc.sync.dma_start(
                    t["out"][:].rearrange("(s p) f -> p s f", p=P)[
                        :, bs[0] : bs[0] + nb, :
                    ],
                    ln2[:, 0:nb],
                )

            return [s0, s1, s2, mk_h(0), mk_h(1), mk_h(2), mk_h(3), s5, s6]

        # ---- chunk list over all groups/blocks ----
        chunks = []
        for g in groups:
            for bi, (b, b0) in enumerate(zip(g["bs"], g["base"])):
                nsb = int(ns[b])
                for ci in range(0, nsb, c.csz):
                    cs = min(c.csz, nsb - ci)
                    chunks.append(
                        dict(
                            g=g, bi=bi, b=b, b0=b0, ci=ci, cs=cs,
                            mtb=b0 // P + ci, nsb=nsb,
                            last_of_group=False,
                        )
                    )
            chunks[-1]["last_of_group"] = True

        def group_setup(g):
            L = g["L"]
            s0 = g["scol"]
            gt = gt_pool.tile([P, 3 * Lmax], BF16, tag="gt")
            nc.sync.dma_start(
                gt[:, 0 : 3 * L], t["SALL"][:, 3 * s0 : 3 * s0 + 3 * L]
            )
            f32 = ep.tile([P, EPB, P], FP32, tag="f32")
            nc.sync.dma_start(
                f32[:, 0 : len(g["bs"]), :],
                t["feat32_loc"][:]
                .rearrange("(s p) f -> p s f", p=P)[
                    :, g["bs"][0] : g["bs"][0] + len(g["bs"]), :
                ],
            )
            g["gt"] = gt
            g["f32"] = f32
            g["Mt"] = mt_pool.tile([P, NSG, 136], BF16, tag="Mt", name="Mt")
            g["misc"] = misc_pool.tile([P, 512], FP32, tag="misc", name="misc")

        def stage_a(ch, idx):
            g = ch["g"]
            gt, L, b0, ci, cs = g["gt"], g["L"], ch["b0"], ch["ci"], ch["cs"]
            kps = k_pool.tile([P, c.csz * P], FP32, tag="kps")
            nc.tensor.matmul(
                kps[:, 0 : cs * P],
                wk[:],
                gt[:, b0 + ci * P : b0 + (ci + cs) * P],
                start=True,
                stop=True,
            )
            qps = q_pool.tile([P, c.csz * P], FP32, tag="qps")
            nc.tensor.matmul(
                qps[:, 0 : cs * P],
                q_sb[:, ch["b"], :],
                gt[:, 2 * L + b0 + ci * P : 2 * L + b0 + (ci + cs) * P],
                start=True,
                stop=True,
            )
            qcp = qcp_pool.tile([P, c.csz * P], BF16, tag="qcp")
            nc.scalar.copy(qcp[:, 0 : cs * P], qps[:, 0 : cs * P])
            tt = tt_pool.tile([P, c.csz, P], BF16, tag="tt")
            nc.vector.tensor_tensor(
                tt[:, 0:cs],
                qcp[:, 0 : cs * P].rearrange("p (s f) -> p s f", f=P),
                kps[:, 0 : cs * P].rearrange("p (s f) -> p s f", f=P),
                op=OP.mult,
            )
            ch["tt"] = tt

        def stage_b1(ch, idx):
            g = ch["g"]
            gt, L, b0, ci, cs, mtb = (
                g["gt"], g["L"], ch["b0"], ch["ci"], ch["cs"], ch["mtb"],
            )
            Mt, misc, tt = g["Mt"], g["misc"], ch["tt"]
            soff = SC0 + (idx % 2) * 32
            nc.vector.memset(misc[:, soff : soff + cs * c.heads], 0.0)
            for s in range(cs):
                nc.tensor.matmul(
                    misc[:, soff + s * c.heads : soff + (s + 1) * c.heads],
                    tt[:, s, :],
                    hsel[:],
                    start=False,
                    stop=True,
                    skip_group_check=True,
                )
            nc.scalar.activation(
                Mt[:, mtb : mtb + cs, 128:136],
                misc[:, soff : soff + cs * c.heads].rearrange(
                    "p (s h) -> p s h", h=c.heads
                ),
                AF.Exp,
            )
            vps = v_pool.tile([P, c.csz, P], FP32, tag="vps")
            for s in range(cs):
                col = b0 + (ci + s) * P
                nc.tensor.matmul(
                    vps[:, s], gt[:, col : col + P], wv[:], start=True, stop=True
                )
            nc.vector.tensor_tensor(
                Mt[:, mtb : mtb + cs, 0:128].rearrange(
                    "p s (h d) -> p s h d", d=c.dhead
                ),
                vps[:, 0:cs].rearrange("p s (h d) -> p s h d", d=c.dhead),
                Mt[:, mtb : mtb + cs, 128:136]
                .rearrange("p s (h o) -> p s h o", o=1)
                .to_broadcast([P, cs, c.heads, c.dhead]),
                op=OP.mult,
            )

        def stage_b2(ch, idx):
            g = ch["g"]
            gt, L, b0, ci, cs, mtb = (
                g["gt"], g["L"], ch["b0"], ch["ci"], ch["cs"], ch["mtb"],
            )
            Mt, misc = g["Mt"], g["misc"]
            for s in range(cs):
                col = b0 + (ci + s) * P
                nc.tensor.matmul(
                    misc[:, ch["bi"] * 136 : ch["bi"] * 136 + 136],
                    gt[:, L + col : L + col + P],
                    Mt[:, mtb + s, :],
                    start=(ci + s == 0),
                    stop=(ci + s == ch["nsb"] - 1),
                    skip_group_check=True,
                )

        # ---- software-pipelined emission ----
        from collections import deque

        n = len(chunks)
        cur_g = None
        pending = deque()
        for i in range(n + 2):
            if i < n:
                ch = chunks[i]
                if ch["g"] is not cur_g:
                    cur_g = ch["g"]
                    group_setup(cur_g)
                stage_a(ch, i)
            if 1 <= i <= n:
                stage_b1(chunks[i - 1], i - 1)
            if 2 <= i <= n + 1:
                ch2 = chunks[i - 2]
                stage_b2(ch2, i - 2)
                if ch2["last_of_group"]:
                    pending.extend(epilogue_stages(ch2["g"]))
            # pop epilogue stages, keeping the backlog to about one group
            if pending:
                pending.popleft()()
            while len(pending) > 7:
                pending.popleft()()
        while pending:
            pending.popleft()()


def _build(meta, cfg: GATCfg):
    c = cfg
    nc = bacc.Bacc(
        "TRN2", target_bir_lowering=False, debug=False, num_devices=c.n_cores
    )
    t = {}

    def inp(name, shape, dtype):
        t[name] = nc.dram_tensor(name, shape, dtype, kind="ExternalInput").ap()

    inp("SALL", [P, 3 * meta["tot_cols"]], BF16)
    inp("feat16_locT", [P, c.local_pad], BF16)
    inp("feat32_loc", [c.local_pad, c.feats], FP32)
    inp("wq", [c.feats, c.feats], BF16)
    inp("wk", [c.feats, c.feats], BF16)
    inp("wv", [c.feats, c.feats], BF16)
    inp("w1", [c.feats, c.dff], BF16)
    inp("w2", [P, c.dff // P, c.feats], BF16)
    inp("b1t", [P, c.dff // P], FP32)
    inp("at", [P, c.dff // P], FP32)
    inp("b2rep", [P, c.feats], FP32)
    inp("grep", [P, c.feats], FP32)
    inp("brep", [P, c.feats], FP32)
    inp("ident", [P, P], BF16)
    inp("hsel", [P, c.heads], BF16)
    t["out"] = nc.dram_tensor(
        "out", [c.local_pad, c.feats], FP32, kind="ExternalOutput"
    ).ap()

    with tile.TileContext(nc) as tc:
        _emit(tc, t, meta, cfg)
    nc.compile()
    return nc


def _in_maps(meta, streams, shared, cfg: GATCfg):
    maps = []
    for ci in range(cfg.n_cores):
        m = dict(shared)
        m.update(streams[ci])
        maps.append(m)
    return maps


_CACHE = {}


def kernel(**inputs) -> np.ndarray:
    cfg = GATCfg()
    meta, streams, shared = _prep(inputs, cfg)
    key = "real"
    if key not in _CACHE:
        _CACHE[key] = _build(meta, cfg)
    nc = _CACHE[key]
    maps = _in_maps(meta, streams, shared, cfg)
    res = run_bass_kernel_spmd(nc, maps, core_ids=list(range(cfg.n_cores)))
    out = np.empty((cfg.n_nodes, cfg.feats), np.float32)
    for ci in range(cfg.n_cores):
        out[ci * cfg.npc : (ci + 1) * cfg.npc] = res.results[ci]["out"][: cfg.npc]
    return out
